# revision 1
# baseline (speedup 1.0000x reference)
"""Sliding-window causal self-attention (GQA + RoPE + QK-RMSNorm + ve-gate) on
8 Trainium2 NeuronCores.

Sharding: core c handles (batch b = c // 4, kv-head g = c % 4): data parallel
over batch x tensor parallel over the 4 KV head groups (4 query heads per
core). Each core computes its partial c_proj output; the all-reduce over the 4
head shards is a host-side sum.

Device design (per core):
  - x is fed transposed (xT: C x T) so all projections contract over the
    partition axis.
  - q, k are built transposed (qT/kT: head-dim x T); scores are computed
    TRANSPOSED (S^T: tk x tq) so softmax denominators come from a ones-matmul
    on the Tensor engine and P@V needs no transposes.
  - softmax skips max-subtraction: QK RMS-norm bounds |scores| <= 1.44*sqrt(128)
    so exp() cannot overflow in fp32. Masking is a -100 bias on the two
    triangular boundary blocks; masked weights underflow to 0.
  - k's rms-norm scale rides the per-partition `scale` operand of the Exp
    activation; q's rides the PSUM-evacuation multiply.
  - all matmuls run in float32r (full PE rate for moving dim >= 256,
    ~1.6e-4 matmul relerr vs fp32).
"""

import sys

sys.path.insert(0, "/opt/trn_rl_repo")

import numpy as np

B, T, C = 2, 2048, 2048
NH, NKV, HD = 16, 4, 128
GATE_CH = 12
HPC = NH // NKV          # q heads per core
TS = 512                 # token-slice width
NSL = T // TS            # 4 slices
NCK = C // 128           # 16 contraction chunks
TPS = TS // 128          # 4 token tiles per slice
NTT = T // 128           # 16 token tiles
EPS = 1e-6
NEG = -100.0

A_Q = 1.2 / np.sqrt(float(HD))   # rms-norm scale folded into q (incl 1/sqrt(HD))
A_K = 1.2                        # rms-norm scale folded into exp() scale arg
S_Q = float(1.0 / (HD * A_Q * A_Q))
B_Q = float(EPS / (A_Q * A_Q))
S_K = float(1.0 / (HD * A_K * A_K))
B_K = float(EPS / (A_K * A_K))

_compiled = {}


def _ktiles(m4, W):
    """k-tiles overlapping q-slice m4 with their valid tq-column extents.

    Returns list of (n, f0, f1, causal_block_col, edge_block_col); columns are
    relative to the slice (0..TS). First entry covers [0, TS) fully (it opens
    the PSUM accumulation group).
    """
    assert W % 128 == 0 and W >= 384
    out = []
    for n in range(0, TPS * m4 + TPS):
        f0 = max(0, 128 * n - TS * m4)
        f1 = min(TS, 128 * n + W + 128 - TS * m4)
        if f1 <= f0:
            continue
        causal = 128 * n >= TS * m4            # diagonal staircase inside tile
        edge = (128 * n + W + 128 - TS * m4) <= TS  # window lower edge inside
        cb = f0 if causal else None
        eb = (f1 - 128) if edge else None
        out.append((n, f0, f1, cb, eb))
    full = [e for e in out if e[1] == 0 and e[2] == TS]
    assert full, "need one full-extent tile to open the PSUM group"
    first = full[0]
    rest = [e for e in out if e[0] != first[0]]
    return [first] + rest


def _build(W):
    import concourse.bass as bass
    import concourse.tile as tile
    from concourse import bacc, mybir
    from concourse.masks import make_identity
    from contextlib import ExitStack

    f32 = mybir.dt.float32
    f32r = mybir.dt.float32r
    AF = mybir.ActivationFunctionType
    OP = mybir.AluOpType

    nc = bacc.Bacc(None, target_bir_lowering=False)

    xT = nc.dram_tensor("xT", [C, T], f32r, kind="ExternalInput")
    wq = nc.dram_tensor("wqT", [C, HPC * HD], f32r, kind="ExternalInput")
    wk = nc.dram_tensor("wkT", [C, HD], f32r, kind="ExternalInput")
    wv = nc.dram_tensor("wvT", [C, HD], f32r, kind="ExternalInput")
    wp = nc.dram_tensor("wpT", [HPC * HD, C], f32r, kind="ExternalInput")
    wgd = nc.dram_tensor("wg", [GATE_CH, 1], f32r, kind="ExternalInput")
    ccd = nc.dram_tensor("cc", [HD, T], f32, kind="ExternalInput")
    ssd = nc.dram_tensor("ss", [HD, T], f32, kind="ExternalInput")
    ved = nc.dram_tensor("ve", [T, HD], f32, kind="ExternalInput")
    btrid = nc.dram_tensor("btri", [128, 128], f32, kind="ExternalInput")
    etrid = nc.dram_tensor("etri", [128, 128], f32, kind="ExternalInput")
    outT = nc.dram_tensor("outT", [C, T], f32, kind="ExternalOutput")

    with tile.TileContext(nc) as tc, ExitStack() as ctx:
        res = ctx.enter_context(tc.tile_pool(name="res", bufs=1))
        xc_p = ctx.enter_context(tc.tile_pool(name="xc", bufs=1))
        tab_p = ctx.enter_context(tc.tile_pool(name="tab", bufs=1))
        work_p = ctx.enter_context(tc.tile_pool(name="work", bufs=2))
        sq_p = ctx.enter_context(tc.tile_pool(name="sq", bufs=3))
        bc_p = ctx.enter_context(tc.tile_pool(name="bc", bufs=2))
        qt_p = ctx.enter_context(tc.tile_pool(name="qt", bufs=2))
        es_p = ctx.enter_context(tc.tile_pool(name="es", bufs=4))
        yt_p = ctx.enter_context(tc.tile_pool(name="yt", bufs=1))
        ot_p = ctx.enter_context(tc.tile_pool(name="ot", bufs=3))
        row_p = ctx.enter_context(tc.tile_pool(name="rows", bufs=1))

        ps_qkv = ctx.enter_context(tc.tile_pool(name="ps_qkv", bufs=2, space="PSUM"))
        ps_s = ctx.enter_context(tc.tile_pool(name="ps_s", bufs=3, space="PSUM"))
        ps_row = ctx.enter_context(tc.tile_pool(name="ps_row", bufs=3, space="PSUM"))
        dram_p = ctx.enter_context(tc.tile_pool(name="dram", bufs=2, space="DRAM"))

        # resident tensors; weight loads split per chunk so the first QKV
        # matmuls can start as soon as their chunk lands (startup latency).
        wq_sb = res.tile([128, NCK, HPC * HD], f32r)
        wk_sb = res.tile([128, NCK, HD], f32r)
        wv_sb = res.tile([128, NCK, HD], f32r)
        wp_sb = res.tile([128, HPC, C], f32r)   # loaded later, before cproj(0)
        wg_sb = res.tile([GATE_CH, 1], f32r)
        nc.sync.dma_start(out=wg_sb, in_=wgd[:, :])
        btri_sb = res.tile([128, 128], f32)
        nc.sync.dma_start(out=btri_sb, in_=btrid[:, :])
        etri_sb = res.tile([128, 128], f32)
        nc.sync.dma_start(out=etri_sb, in_=etrid[:, :])
        ident = res.tile([128, 128], f32)
        make_identity(nc, ident)
        ones_f = res.tile([128, 1], f32)
        nc.vector.memset(ones_f, 1.0)
        ones_sb = ones_f.bitcast(f32r)
        bq_sb = res.tile([1, 1], f32)
        nc.vector.memset(bq_sb, B_Q)
        bk_sb = res.tile([128, 1], f32)
        nc.vector.memset(bk_sb, B_K)
        kT_sb = res.tile([128, T], f32r)        # rotated k, head-dim on partitions
        vn_sb = res.tile([128, NTT, HD], f32r)  # v natural, token tiles on partitions
        rnk_sb = res.tile([128, NTT], f32)      # per-k-tile rms-norm columns

        def rope_inplace(dst, cc_sl, ss_sl):
            """dst (128, TS) f32r holding pre-rotation values. In-place RoPE."""
            qsw = work_p.tile([128, TS], f32, tag="qsw")
            nc.sync.dma_start(out=qsw[0:64, :], in_=dst[64:128, :].bitcast(f32))
            nc.sync.dma_start(out=qsw[64:128, :], in_=dst[0:64, :].bitcast(f32))
            tmp = work_p.tile([128, TS], f32, tag="tmp")
            nc.gpsimd.tensor_mul(tmp, qsw, ss_sl)
            nc.vector.tensor_mul(dst, dst.bitcast(f32), cc_sl)
            nc.vector.tensor_add(dst, dst.bitcast(f32), tmp)

        for m4 in range(NSL):
            t0 = m4 * TS
            # ---- stream x slice + tables ----
            xc = []
            for c in range(NCK):
                xt = xc_p.tile([128, TS], f32r, tag=f"xc{c}")
                nc.sync.dma_start(out=xt, in_=xT[c * 128:(c + 1) * 128, t0:t0 + TS])
                xc.append(xt)
                if m4 == 0:
                    nc.sync.dma_start(out=wk_sb[:, c, :],
                                      in_=wk[c * 128:(c + 1) * 128, :])
            cc_sl = tab_p.tile([128, TS], f32, tag="cc")
            nc.sync.dma_start(out=cc_sl, in_=ccd[:, t0:t0 + TS])
            ss_sl = tab_p.tile([128, TS], f32, tag="ss")
            nc.sync.dma_start(out=ss_sl, in_=ssd[:, t0:t0 + TS])
            ve_sl = tab_p.tile([128, TPS, HD], f32, tag="ve")
            nc.sync.dma_start(
                out=ve_sl, in_=ved[t0:t0 + TS, :].rearrange("(tt p) h -> p tt h", p=128)
            )

            # ---- gate columns: 3*sigmoid(x[:, :12] @ wg) ----
            ps_g = ps_row.tile([1, TS], f32, tag="rows")
            nc.tensor.matmul(ps_g, wg_sb, xc[0][0:GATE_CH, :], start=True, stop=True)
            g_row = row_p.tile([1, TS], f32, tag="grow")
            nc.scalar.activation(g_row, ps_g, AF.Exp, scale=-1.0)
            nc.vector.tensor_scalar(out=g_row, in0=g_row, scalar1=1.0, scalar2=None,
                                    op0=OP.add)
            nc.vector.reciprocal(g_row, g_row)
            g_dr = dram_p.tile([TS], f32, tag="gdr")
            nc.sync.dma_start(out=g_dr, in_=g_row)
            gate_c = row_p.tile([128, TPS], f32, tag="gate")
            nc.sync.dma_start(
                out=gate_c,
                in_=bass.AP(tensor=g_dr.tensor, offset=g_dr.offset,
                            ap=[[1, 128], [128, TPS]]),
            )

            # ---- k projection + rms-norm cols + rope ----
            ps_k = ps_qkv.tile([128, TS], f32, tag="qkv")
            for c in range(NCK):
                nc.tensor.matmul(ps_k, wk_sb[:, c, :], xc[c],
                                 start=(c == 0), stop=(c == NCK - 1))
            sq_k = sq_p.tile([128, TS], f32r, tag="sq")
            nc.scalar.activation(sq_k, ps_k, AF.Square)
            ps_rk = ps_row.tile([1, TS], f32, tag="rows")
            nc.tensor.matmul(ps_rk, ones_sb, sq_k, start=True, stop=True)
            srk = row_p.tile([1, TS], f32, tag="srk")
            nc.scalar.activation(srk, ps_rk, AF.Ln, bias=bk_sb[0:1], scale=S_K)
            nc.scalar.activation(srk, srk, AF.Exp, scale=-0.5)
            k_dr = dram_p.tile([TS], f32, tag="kdr")
            nc.sync.dma_start(out=k_dr, in_=srk)
            nc.sync.dma_start(
                out=rnk_sb[:, m4 * TPS:(m4 + 1) * TPS],
                in_=bass.AP(tensor=k_dr.tensor, offset=k_dr.offset,
                            ap=[[1, 128], [128, TPS]]),
            )
            k_sl = kT_sb[:, t0:t0 + TS]
            nc.vector.tensor_copy(k_sl, ps_k)
            rope_inplace(k_sl, cc_sl, ss_sl)

            # ---- v projection + transpose to natural + gate-add ----
            if m4 == 0:
                for c in range(NCK):
                    nc.sync.dma_start(out=wv_sb[:, c, :],
                                      in_=wv[c * 128:(c + 1) * 128, :])
            ps_v = ps_qkv.tile([128, TS], f32, tag="qkv")
            for c in range(NCK):
                nc.tensor.matmul(ps_v, wv_sb[:, c, :], xc[c],
                                 start=(c == 0), stop=(c == NCK - 1))
            vT_s = work_p.tile([128, TS], f32, tag="qsw")
            nc.vector.tensor_copy(vT_s, ps_v)
            for tt in range(TPS):
                ps_t = ps_s.tile([128, TS], f32, tag="s")
                nc.tensor.transpose(ps_t[:, 0:128], vT_s[:, tt * 128:(tt + 1) * 128],
                                    ident)
                gtmp = work_p.tile([128, HD], f32, tag="gtmp")
                nc.vector.tensor_scalar(out=gtmp, in0=ve_sl[:, tt, :],
                                        scalar1=gate_c[:, tt:tt + 1], scalar2=3.0,
                                        op0=OP.mult, op1=OP.mult)
                nc.vector.tensor_add(vn_sb[:, m4 * TPS + tt, :], ps_t[:, 0:128], gtmp)

            # ---- q projections (4 heads) + rms-norm + rope ----
            if m4 == 0:
                for c in range(NCK):
                    nc.sync.dma_start(out=wq_sb[:, c, :],
                                      in_=wq[c * 128:(c + 1) * 128, :])
            qts = []
            for h in range(HPC):
                ps_q = ps_qkv.tile([128, TS], f32, tag="qkv")
                for c in range(NCK):
                    nc.tensor.matmul(ps_q, wq_sb[:, c, h * HD:(h + 1) * HD], xc[c],
                                     start=(c == 0), stop=(c == NCK - 1))
                sq_q = sq_p.tile([128, TS], f32r, tag="sq")
                nc.scalar.activation(sq_q, ps_q, AF.Square)
                ps_r = ps_row.tile([1, TS], f32, tag="rows")
                nc.tensor.matmul(ps_r, ones_sb, sq_q, start=True, stop=True)
                srow = row_p.tile([1, TS], f32, tag="srow")
                nc.scalar.activation(srow, ps_r, AF.Ln, bias=bq_sb, scale=S_Q)
                nc.scalar.activation(srow, srow, AF.Exp, scale=-0.5)
                rbc = bc_p.tile([128, TS], f32, tag="bc")
                nc.gpsimd.partition_broadcast(rbc, srow)
                qt = qt_p.tile([128, TS], f32r, tag=f"qt{h}")
                nc.vector.tensor_mul(qt, ps_q, rbc)
                rope_inplace(qt, cc_sl, ss_sl)
                qts.append(qt)

            # ---- attention (scores transposed: tk on partitions, tq free) ----
            tiles = _ktiles(m4, W)
            last = len(tiles) - 1
            yts = []
            for h in range(HPC):
                ps_out = ps_row.tile([128, TS], f32, tag="rows")
                ps_sum = ps_row.tile([1, TS], f32, tag="rows")
                for idx, (n, f0, f1, cb, eb) in enumerate(tiles):
                    pss = ps_s.tile([128, TS], f32, tag="s")
                    nc.tensor.matmul(pss[:, f0:f1], kT_sb[:, n * 128:(n + 1) * 128],
                                     qts[h][:, f0:f1], start=True, stop=True)
                    es = es_p.tile([128, TS], f32r, tag="es")
                    nc.scalar.activation(es[:, f0:f1], pss[:, f0:f1], AF.Exp,
                                         scale=rnk_sb[:, n:n + 1])
                    if cb is not None:
                        nc.gpsimd.tensor_mul(es[:, cb:cb + 128],
                                             es[:, cb:cb + 128].bitcast(f32), btri_sb)
                    if eb is not None:
                        nc.gpsimd.tensor_mul(es[:, eb:eb + 128],
                                             es[:, eb:eb + 128].bitcast(f32), etri_sb)
                    nc.tensor.matmul(ps_sum[:, f0:f1], ones_sb, es[:, f0:f1],
                                     start=(idx == 0), stop=(idx == last))
                    nc.tensor.matmul(ps_out[:, f0:f1], vn_sb[:, n, :], es[:, f0:f1],
                                     start=(idx == 0), stop=(idx == last))
                rsum = row_p.tile([1, TS], f32, tag="rsum")
                nc.vector.reciprocal(rsum, ps_sum)
                sbc = bc_p.tile([128, TS], f32, tag="bc")
                nc.gpsimd.partition_broadcast(sbc, rsum)
                yt = yt_p.tile([128, TS], f32r, tag=f"yt{h}")
                nc.vector.tensor_mul(yt, ps_out, sbc)
                yts.append(yt)

            # ---- c_proj partial: outT[co, t] = sum_h wpT[h].T @ yT[h] ----
            if m4 == 0:
                for h in range(HPC):
                    nc.sync.dma_start(out=wp_sb[:, h, :],
                                      in_=wp[h * 128:(h + 1) * 128, :])
            for co in range(NTT):
                ps_p = ps_s.tile([128, TS], f32, tag="s")
                for h in range(HPC):
                    nc.tensor.matmul(ps_p, wp_sb[:, h, co * 128:(co + 1) * 128],
                                     yts[h], start=(h == 0), stop=(h == HPC - 1))
                ot = ot_p.tile([128, TS], f32, tag="ot")
                nc.vector.tensor_copy(ot, ps_p)
                nc.sync.dma_start(out=outT[co * 128:(co + 1) * 128, t0:t0 + TS],
                                  in_=ot)

    # Restrict the activation-table picker to the one set containing every
    # ACT function we use (exp, ln, square, copy, identity): without this the
    # greedy picker alternates exp_and_others <-> natural_log, inserting a
    # ~1.3us table load per switch. Set ids are positions in act_info.json's
    # list, so unwanted sets are emptied rather than removed.
    import concourse.hw_specs as hw_specs
    import concourse.bacc as bacc_mod

    orig = hw_specs.get_activation_tables

    def only_combined(arch):
        t = orig(arch)
        return {k: (v if k == "natural_log_exp_and_others" else set())
                for k, v in t.items()}

    hw_specs.get_activation_tables = only_combined
    bacc_mod.get_activation_tables = only_combined
    try:
        nc.compile()
    finally:
        hw_specs.get_activation_tables = orig
        bacc_mod.get_activation_tables = orig
    return nc


def _prep_inputs(x, ve, cos, sin, Wq, Wk, Wv, Wproj, Wgate, W):
    cosT = np.ascontiguousarray(cos[0, :, 0, :].T)  # (64, T)
    sinT = np.ascontiguousarray(sin[0, :, 0, :].T)
    cc = np.concatenate([cosT, cosT], axis=0).astype(np.float32)
    ss = np.concatenate([sinT, -sinT], axis=0).astype(np.float32)
    p = np.arange(128)[:, None]
    f = np.arange(128)[None, :]
    btri = (p <= f).astype(np.float32)
    etri = (f <= p + (W % 128)).astype(np.float32)

    in_maps = []
    for core in range(8):
        b, g = core // NKV, core % NKV
        hs = slice(g * HPC * HD, (g + 1) * HPC * HD)
        ks = slice(g * HD, (g + 1) * HD)
        in_maps.append({
            "xT": np.ascontiguousarray(x[b].T),
            "wqT": np.ascontiguousarray(Wq[hs, :].T),
            "wkT": np.ascontiguousarray(Wk[ks, :].T),
            "wvT": np.ascontiguousarray(Wv[ks, :].T),
            "wpT": np.ascontiguousarray(Wproj[:, hs].T),
            "wg": np.ascontiguousarray(Wgate[g][:, None]),
            "cc": cc,
            "ss": ss,
            "ve": np.ascontiguousarray(ve[b][:, ks]),
            "btri": btri,
            "etri": etri,
        })
    return in_maps


def _run(inputs, trace=False):
    from concourse.bass_utils import run_bass_kernel_spmd

    x = np.asarray(inputs["x"], dtype=np.float32)
    ve = np.asarray(inputs["ve"], dtype=np.float32)
    cos = np.asarray(inputs["cos"], dtype=np.float32)
    sin = np.asarray(inputs["sin"], dtype=np.float32)
    Wq = np.asarray(inputs["Wq"], dtype=np.float32)
    Wk = np.asarray(inputs["Wk"], dtype=np.float32)
    Wv = np.asarray(inputs["Wv"], dtype=np.float32)
    Wproj = np.asarray(inputs["Wproj"], dtype=np.float32)
    Wgate = np.asarray(inputs["Wgate"], dtype=np.float32)
    W = int(inputs["window_size"])

    if W not in _compiled:
        _compiled[W] = _build(W)
    nc = _compiled[W]

    in_maps = _prep_inputs(x, ve, cos, sin, Wq, Wk, Wv, Wproj, Wgate, W)
    res = run_bass_kernel_spmd(nc, in_maps, core_ids=list(range(8)), trace=trace)

    out = np.zeros((B, T, C), dtype=np.float32)
    for core in range(8):
        b = core // NKV
        out[b] += res.results[core]["outT"].T
    return out, res


def kernel(**inputs):
    out, _ = _run(inputs, trace=False)
    return out



# revision 4
# speedup vs baseline: 1.2399x; 1.2399x over previous
"""Sliding-window causal self-attention (GQA + RoPE + QK-RMSNorm + ve-gate) on
8 Trainium2 NeuronCores.

Sharding: core c handles (batch b = c // 4, kv-head g = c % 4): data parallel
over batch x tensor parallel over the 4 KV head groups (4 query heads per
core). Each core computes its partial c_proj output; the all-reduce over the 4
head shards is a host-side sum.

v2 design (per core):
  - everything the PE touches is bf16 (inputs are host-converted); PSUM
    accumulation stays fp32, so matmul error is input-quantization only
    (~0.2% rms per stage, ~60x under the 2e-2 gate).
  - the ve gate (3*sigmoid(x[:,:12] @ Wgate)) is folded into ve on the host:
    ve' = gate * ve, so the device only does v += ve'.
  - k's rms-norm is folded into kT right at PSUM evacuation (broadcast row *
    PSUM), so exp() needs no per-key scale and there are no DRAM round trips.
  - v is computed directly in natural (token-partition) layout by using the
    x chunk as the matmul stationary operand: no PE transposes.
  - RoPE's half-swap uses DVE reads at a shifted partition base instead of
    SBUF->SBUF DMAs.
  - scores are computed transposed (S^T: tk x tq); softmax denominators come
    from a ones-stationary matmul; no max-subtraction (QK rms-norm bounds
    |score| <= 1.44*sqrt(128)); masking multiplies boundary tiles by 0/1
    triangles on the Pool engine.
  - DMA count is ~41 (vs 251): weights/tables are host-prepacked into SBUF
    layout ([128, free]) so each is one large-descriptor DMA; x streams in 4
    group-DMAs per 512-token slice; output streams out in 4 group-DMAs per
    slice (bf16 partials, host sums in fp32).
  - the slice loop is software-pipelined: k/v projections of slice m+1 are
    issued between attention(m) and c_proj(m) so the PE never waits on the
    rms/rope latency chains.
"""

import sys

sys.path.insert(0, "/opt/trn_rl_repo")

import numpy as np

B, T, C = 2, 2048, 2048
NH, NKV, HD = 16, 4, 128
GATE_CH = 12
HPC = NH // NKV          # q heads per core
TS = 512                 # token-slice width
NSL = T // TS            # 4 slices
NCK = C // 128           # 16 contraction chunks
TPS = TS // 128          # 4 token tiles per slice
NTT = T // 128           # 16 token tiles
EPS = 1e-6

A_Q = 1.2 / np.sqrt(float(HD))   # rms-norm scale folded into q (incl 1/sqrt(HD))
A_K = 1.2                        # rms-norm scale folded into k
S_Q = float(1.0 / (HD * A_Q * A_Q))
B_Q = float(EPS / (A_Q * A_Q))
S_K = float(1.0 / (HD * A_K * A_K))
B_K = float(EPS / (A_K * A_K))

_compiled = {}


def _ktiles(m4, W):
    """k-tiles overlapping q-slice m4 with their valid tq-column extents.

    Returns list of (n, f0, f1, causal_block_col, edge_block_col); columns are
    relative to the slice (0..TS). First entry covers [0, TS) fully (it opens
    the PSUM accumulation group).
    """
    assert W % 128 == 0 and W >= 384
    out = []
    for n in range(0, TPS * m4 + TPS):
        f0 = max(0, 128 * n - TS * m4)
        f1 = min(TS, 128 * n + W + 128 - TS * m4)
        if f1 <= f0:
            continue
        causal = 128 * n >= TS * m4            # diagonal staircase inside tile
        edge = (128 * n + W + 128 - TS * m4) <= TS  # window lower edge inside
        cb = f0 if causal else None
        eb = (f1 - 128) if edge else None
        out.append((n, f0, f1, cb, eb))
    full = [e for e in out if e[1] == 0 and e[2] == TS]
    assert full, "need one full-extent tile to open the PSUM group"
    first = full[0]
    rest = [e for e in out if e[0] != first[0]]
    return [first] + rest


def _build(W):
    import concourse.bass as bass
    import concourse.tile as tile
    from concourse import bacc, mybir
    from contextlib import ExitStack

    f32 = mybir.dt.float32
    bf16 = mybir.dt.bfloat16
    AF = mybir.ActivationFunctionType

    nc = bacc.Bacc(None, target_bir_lowering=False)

    xd = nc.dram_tensor("xp", [128, NCK, T], bf16, kind="ExternalInput")
    wqd = nc.dram_tensor("wqp", [128, NCK * HPC * HD], bf16, kind="ExternalInput")
    wkd = nc.dram_tensor("wkp", [128, NCK * HD], bf16, kind="ExternalInput")
    wvd = nc.dram_tensor("wvp", [128, NCK * HD], bf16, kind="ExternalInput")
    wpd = nc.dram_tensor("wpp", [128, HPC * C], bf16, kind="ExternalInput")
    ccd = nc.dram_tensor("cc", [128, T], bf16, kind="ExternalInput")
    ssd = nc.dram_tensor("ss", [128, T], bf16, kind="ExternalInput")
    ved = nc.dram_tensor("vep", [128, NTT * HD], bf16, kind="ExternalInput")
    trid = nc.dram_tensor("tri", [128, 256], bf16, kind="ExternalInput")
    outd = nc.dram_tensor("outp", [128, NTT, T], bf16, kind="ExternalOutput")

    with tile.TileContext(nc) as tc, ExitStack() as ctx:
        res = ctx.enter_context(tc.tile_pool(name="res", bufs=1))
        xc_p = ctx.enter_context(tc.tile_pool(name="xc", bufs=2))
        sq_p = ctx.enter_context(tc.tile_pool(name="sq", bufs=2))
        row_p = ctx.enter_context(tc.tile_pool(name="rows", bufs=2))
        bc_p = ctx.enter_context(tc.tile_pool(name="bc", bufs=3))
        qt_p = ctx.enter_context(tc.tile_pool(name="qt", bufs=2))
        es_p = ctx.enter_context(tc.tile_pool(name="es", bufs=4))
        yt_p = ctx.enter_context(tc.tile_pool(name="yt", bufs=2))
        work_p = ctx.enter_context(tc.tile_pool(name="work", bufs=2))
        ot_p = ctx.enter_context(tc.tile_pool(name="ot", bufs=2))

        ps_qkv = ctx.enter_context(tc.tile_pool(name="ps_qkv", bufs=2, space="PSUM"))
        ps_s = ctx.enter_context(tc.tile_pool(name="ps_s", bufs=3, space="PSUM"))
        ps_row = ctx.enter_context(tc.tile_pool(name="ps_row", bufs=3, space="PSUM"))

        # ---- resident tensors ----
        wq_sb = res.tile([128, NCK, HPC * HD], bf16)
        wk_sb = res.tile([128, NCK, HD], bf16)
        wv_sb = res.tile([128, NCK, HD], bf16)
        wp_sb = res.tile([128, HPC, C], bf16)
        cc_sb = res.tile([128, T], bf16)
        ss_sb = res.tile([128, T], bf16)
        ve_sb = res.tile([128, NTT, HD], bf16)
        tri_sb = res.tile([128, 256], bf16)
        kT_sb = res.tile([128, T], bf16)        # rotated+normalized k, HD on partitions
        vn_sb = res.tile([128, NTT, HD], bf16)  # v natural, token tiles on partitions
        ones_sb = res.tile([128, 1], bf16)
        nc.vector.memset(ones_sb, 1.0)
        bq_sb = res.tile([1, 1], f32)
        nc.vector.memset(bq_sb, B_Q)
        bk_sb = res.tile([1, 1], f32)
        nc.vector.memset(bk_sb, B_K)

        xcs = {}

        def emit_xload(m4):
            xc = xc_p.tile([128, NCK, TS], bf16, tag="xc")
            t0 = m4 * TS
            for g in range(4):
                nc.sync.dma_start(out=xc[:, 4 * g:4 * g + 4, :],
                                  in_=xd[:, 4 * g:4 * g + 4, t0:t0 + TS])
            xcs[m4] = xc

        def rope(dst, t0):
            """In-place RoPE on a [128, TS] bf16 tile; halves swap via DVE
            reads at a shifted partition base."""
            tmp = work_p.tile([128, TS], bf16, tag="tmp")
            nc.vector.tensor_mul(tmp[0:64, :], dst[64:128, :], ss_sb[64:128, t0:t0 + TS])
            nc.vector.tensor_mul(tmp[64:128, :], dst[0:64, :], ss_sb[0:64, t0:t0 + TS])
            nc.vector.tensor_mul(dst, dst, cc_sb[:, t0:t0 + TS])
            nc.vector.tensor_add(dst, dst, tmp)

        def emit_kvproj(m4):
            t0 = m4 * TS
            xc = xcs[m4]
            # k projection + folded rms-norm + rope
            ps_k = ps_qkv.tile([128, TS], f32, tag="qkv")
            for c in range(NCK):
                nc.tensor.matmul(ps_k, wk_sb[:, c, :], xc[:, c, :],
                                 start=(c == 0), stop=(c == NCK - 1))
            sq_k = sq_p.tile([128, TS], bf16, tag="sq")
            nc.scalar.activation(sq_k, ps_k, AF.Square)
            ps_rk = ps_row.tile([1, TS], f32, tag="rows")
            nc.tensor.matmul(ps_rk, ones_sb, sq_k, start=True, stop=True)
            srk = row_p.tile([1, TS], f32, tag="srk")
            nc.scalar.activation(srk, ps_rk, AF.Ln, bias=bk_sb, scale=S_K)
            nc.scalar.activation(srk, srk, AF.Exp, scale=-0.5)
            rbk = bc_p.tile([128, TS], f32, tag="bc")
            nc.gpsimd.partition_broadcast(rbk, srk)
            k_sl = kT_sb[:, t0:t0 + TS]
            nc.vector.tensor_mul(k_sl, ps_k, rbk)
            rope(k_sl, t0)
            # v projection straight into natural layout (x chunk stationary),
            # then the host-pregated ve add
            ps_v = ps_qkv.tile([128, TPS, HD], f32, tag="qkv")
            for tt in range(TPS):
                for c in range(NCK):
                    nc.tensor.matmul(ps_v[:, tt, :], xc[:, c, tt * 128:(tt + 1) * 128],
                                     wv_sb[:, c, :], start=(c == 0), stop=(c == NCK - 1))
                nc.vector.tensor_add(vn_sb[:, m4 * TPS + tt, :], ps_v[:, tt, :],
                                     ve_sb[:, m4 * TPS + tt, :])

        def emit_qproj(m4):
            t0 = m4 * TS
            xc = xcs[m4]
            qts = []
            for h in range(HPC):
                ps_q = ps_qkv.tile([128, TS], f32, tag="qkv")
                for c in range(NCK):
                    nc.tensor.matmul(ps_q, wq_sb[:, c, h * HD:(h + 1) * HD], xc[:, c, :],
                                     start=(c == 0), stop=(c == NCK - 1))
                sq_q = sq_p.tile([128, TS], bf16, tag="sq")
                nc.scalar.activation(sq_q, ps_q, AF.Square)
                ps_r = ps_row.tile([1, TS], f32, tag="rows")
                nc.tensor.matmul(ps_r, ones_sb, sq_q, start=True, stop=True)
                srow = row_p.tile([1, TS], f32, tag="srow")
                nc.scalar.activation(srow, ps_r, AF.Ln, bias=bq_sb, scale=S_Q)
                nc.scalar.activation(srow, srow, AF.Exp, scale=-0.5)
                rbc = bc_p.tile([128, TS], f32, tag="bc")
                nc.gpsimd.partition_broadcast(rbc, srow)
                qt = qt_p.tile([128, TS], bf16, tag=f"qt{h}")
                nc.vector.tensor_mul(qt, ps_q, rbc)
                rope(qt, t0)
                qts.append(qt)
            return qts

        def emit_attn(m4, qts):
            tiles = _ktiles(m4, W)
            last = len(tiles) - 1
            yts = []
            for h in range(HPC):
                ps_out = ps_row.tile([128, TS], f32, tag="rows")
                ps_sum = ps_row.tile([1, TS], f32, tag="rows")
                for idx, (n, f0, f1, cb, eb) in enumerate(tiles):
                    pss = ps_s.tile([128, TS], f32, tag="s")
                    nc.tensor.matmul(pss[:, f0:f1], kT_sb[:, n * 128:(n + 1) * 128],
                                     qts[h][:, f0:f1], start=True, stop=True)
                    es = es_p.tile([128, TS], bf16, tag="es")
                    nc.scalar.activation(es[:, f0:f1], pss[:, f0:f1], AF.Exp)
                    if cb is not None:
                        nc.gpsimd.tensor_mul(es[:, cb:cb + 128], es[:, cb:cb + 128],
                                             tri_sb[:, 0:128])
                    if eb is not None:
                        nc.gpsimd.tensor_mul(es[:, eb:eb + 128], es[:, eb:eb + 128],
                                             tri_sb[:, 128:256])
                    nc.tensor.matmul(ps_sum[:, f0:f1], ones_sb, es[:, f0:f1],
                                     start=(idx == 0), stop=(idx == last))
                    nc.tensor.matmul(ps_out[:, f0:f1], vn_sb[:, n, :], es[:, f0:f1],
                                     start=(idx == 0), stop=(idx == last))
                rsum = row_p.tile([1, TS], f32, tag="rsum")
                nc.vector.reciprocal(rsum, ps_sum)
                sbc = bc_p.tile([128, TS], f32, tag="bc")
                nc.gpsimd.partition_broadcast(sbc, rsum)
                yt = yt_p.tile([128, TS], bf16, tag=f"yt{h}")
                nc.vector.tensor_mul(yt, ps_out, sbc)
                yts.append(yt)
            return yts

        def emit_cproj(m4, yts):
            t0 = m4 * TS
            for cg in range(4):
                ot = ot_p.tile([128, 4, TS], bf16, tag="ot")
                for j in range(4):
                    co = cg * 4 + j
                    ps_p = ps_s.tile([128, TS], f32, tag="s")
                    for h in range(HPC):
                        nc.tensor.matmul(ps_p, wp_sb[:, h, co * 128:(co + 1) * 128],
                                         yts[h], start=(h == 0), stop=(h == HPC - 1))
                    nc.vector.tensor_copy(ot[:, j, :], ps_p)
                nc.sync.dma_start(out=outd[:, 4 * cg:4 * cg + 4, t0:t0 + TS], in_=ot)

        # ---- prologue: weight/table loads ordered by first use ----
        nc.sync.dma_start(out=wk_sb, in_=wkd[:, :].rearrange("p (c h) -> p c h", c=NCK))
        nc.sync.dma_start(out=wv_sb, in_=wvd[:, :].rearrange("p (c h) -> p c h", c=NCK))
        emit_xload(0)
        nc.sync.dma_start(out=cc_sb, in_=ccd[:, :])
        nc.sync.dma_start(out=ss_sb, in_=ssd[:, :])
        nc.sync.dma_start(out=ve_sb, in_=ved[:, :].rearrange("p (n h) -> p n h", n=NTT))
        nc.sync.dma_start(out=tri_sb, in_=trid[:, :])
        nc.sync.dma_start(out=wq_sb, in_=wqd[:, :].rearrange("p (c h) -> p c h", c=NCK))
        nc.sync.dma_start(out=wp_sb, in_=wpd[:, :].rearrange("p (g o) -> p g o", g=HPC))

        # ---- software-pipelined slice loop ----
        emit_kvproj(0)
        for m4 in range(NSL):
            if m4 + 1 < NSL:
                emit_xload(m4 + 1)
            qts = emit_qproj(m4)
            yts = emit_attn(m4, qts)
            if m4 + 1 < NSL:
                emit_kvproj(m4 + 1)
            emit_cproj(m4, yts)

    # Restrict the activation-table picker to the one set containing every
    # ACT function we use (exp, ln, square, copy, identity): without this the
    # greedy picker alternates exp_and_others <-> natural_log, inserting a
    # ~1.3us table load per switch. Set ids are positions in act_info.json's
    # list, so unwanted sets are emptied rather than removed.
    import concourse.hw_specs as hw_specs
    import concourse.bacc as bacc_mod

    orig = hw_specs.get_activation_tables

    def only_combined(arch):
        t = orig(arch)
        return {k: (v if k == "natural_log_exp_and_others" else set())
                for k, v in t.items()}

    hw_specs.get_activation_tables = only_combined
    bacc_mod.get_activation_tables = only_combined
    try:
        nc.compile()
    finally:
        hw_specs.get_activation_tables = orig
        bacc_mod.get_activation_tables = orig
    return nc


def _pack_rows(a):
    """[C_rows, F] -> [128, (C_rows/128) * F] SBUF layout (partition-major)."""
    from ml_dtypes import bfloat16

    rows, f = a.shape
    nck = rows // 128
    return np.ascontiguousarray(
        a.reshape(nck, 128, f).transpose(1, 0, 2).reshape(128, nck * f)
    ).astype(bfloat16)


def _prep_inputs(x, ve, cos, sin, Wq, Wk, Wv, Wproj, Wgate, W):
    from ml_dtypes import bfloat16

    cosT = cos[0, :, 0, :].T  # (64, T)
    sinT = sin[0, :, 0, :].T
    cc = np.concatenate([cosT, cosT], axis=0).astype(bfloat16)
    # rows 0:64 = -sin (used by tmp[64:128] = x1 * -sin, both operands base 0),
    # rows 64:128 = sin (used by tmp[0:64] = x2 * sin, both operands base 64):
    # the BIR verifier requires both SBUF inputs of a TensorTensor to share a
    # partition base, so the table is laid out to match dst's read base.
    ss = np.concatenate([-sinT, sinT], axis=0).astype(bfloat16)
    p = np.arange(128)[:, None]
    f = np.arange(128)[None, :]
    btri = (p <= f).astype(np.float32)
    etri = (f <= p + (W % 128)).astype(np.float32)
    tri = np.concatenate([btri, etri], axis=1).astype(bfloat16)

    xp_by_b = {}
    for b in range(B):
        xp_by_b[b] = _pack_rows(x[b].T)  # [128, NCK*T]

    in_maps = []
    for core in range(8):
        b, g = core // NKV, core % NKV
        hs = slice(g * HPC * HD, (g + 1) * HPC * HD)
        ks = slice(g * HD, (g + 1) * HD)
        gate = 3.0 / (1.0 + np.exp(-(x[b][:, :GATE_CH] @ Wgate[g])))  # (T,)
        vep = gate[:, None] * ve[b][:, ks]  # (T, HD)
        in_maps.append({
            "xp": xp_by_b[b],
            "wqp": _pack_rows(Wq[hs, :].T),
            "wkp": _pack_rows(Wk[ks, :].T),
            "wvp": _pack_rows(Wv[ks, :].T),
            "wpp": _pack_rows(Wproj[:, hs].T),
            "cc": cc,
            "ss": ss,
            "vep": _pack_rows(vep),
            "tri": tri,
        })
    return in_maps


def _run(inputs, trace=False):
    from concourse.bass_utils import run_bass_kernel_spmd

    x = np.asarray(inputs["x"], dtype=np.float32)
    ve = np.asarray(inputs["ve"], dtype=np.float32)
    cos = np.asarray(inputs["cos"], dtype=np.float32)
    sin = np.asarray(inputs["sin"], dtype=np.float32)
    Wq = np.asarray(inputs["Wq"], dtype=np.float32)
    Wk = np.asarray(inputs["Wk"], dtype=np.float32)
    Wv = np.asarray(inputs["Wv"], dtype=np.float32)
    Wproj = np.asarray(inputs["Wproj"], dtype=np.float32)
    Wgate = np.asarray(inputs["Wgate"], dtype=np.float32)
    W = int(inputs["window_size"])

    if W not in _compiled:
        _compiled[W] = _build(W)
    nc = _compiled[W]

    in_maps = _prep_inputs(x, ve, cos, sin, Wq, Wk, Wv, Wproj, Wgate, W)
    res = run_bass_kernel_spmd(nc, in_maps, core_ids=list(range(8)), trace=trace)

    out = np.zeros((B, T, C), dtype=np.float32)
    for core in range(8):
        b = core // NKV
        # outp: [128, NTT, T] with (p, co, t) = partial[co*128+p, t]
        part = np.asarray(res.results[core]["outp"]).astype(np.float32)
        out[b] += part.transpose(1, 0, 2).reshape(C, T).T
    return out, res


def kernel(**inputs):
    out, _ = _run(inputs, trace=False)
    return out


# revision 12
# speedup vs baseline: 1.2607x; 1.0168x over previous
"""Sliding-window causal self-attention (GQA + RoPE + QK-RMSNorm + ve-gate) on
8 Trainium2 NeuronCores.

Sharding: core c handles (batch b = c // 4, kv-head g = c % 4): data parallel
over batch x tensor parallel over the 4 KV head groups (4 query heads per
core). Each core computes its partial c_proj output; the all-reduce over the 4
head shards is a host-side sum.

v3 design (per core):
  - everything the PE touches is bf16 (inputs are host-converted); PSUM
    accumulation stays fp32, so matmul error is input-quantization only.
  - the ve gate (3*sigmoid(x[:,:12] @ Wgate)) is folded into ve on the host:
    ve' = gate * ve, so the device only does v += ve'.
  - k's rms-norm is folded into kT at PSUM evacuation (broadcast row * PSUM),
    so exp() needs no per-key scale and there are no DRAM round trips.
  - v is computed directly in natural (token-partition) layout by using the
    x chunk as the matmul stationary operand: no PE transposes. The k and v
    chunk loops are fused so slice-0 projections track the x DMA arrivals.
  - RoPE's half-swap uses DVE reads at a shifted partition base (the ss table
    is laid out [-sin; sin] so both SBUF inputs of each TensorTensor share a
    partition base, which the BIR verifier requires).
  - scores are computed transposed (S^T: tk x tq); softmax denominators come
    from a ones-stationary matmul into a shared [97, TS] PSUM tile (rows at
    32h: matmul outputs must start at partition 0/32/64/96); no
    max-subtraction (QK rms-norm bounds |score| <= 1.44*sqrt(128)); masking
    multiplies boundary tiles by 0/1 triangles on the Pool engine.
  - c_proj of slice m-1 is interleaved as single-matmul fillers between the
    attention tiles of slice m: the in-order PE would otherwise park at
    sum(i) waiting for exp(i) on the ACT engine (ACT is 2x slower per column
    than the PE).
  - q-head projections alternate between two PSUM pools so the
    square->rownorm->broadcast->evac chain of head h never blocks head h+1.
  - DMA count is ~41 (vs 251 in the original): weights/tables are
    host-prepacked into SBUF layout ([128, free]) so each is one
    large-descriptor DMA; x streams in 4 group-DMAs per 512-token slice;
    output streams out in 4 group-DMAs per slice (bf16 partials, host sums
    in fp32).
"""

import sys

sys.path.insert(0, "/opt/trn_rl_repo")

import numpy as np

B, T, C = 2, 2048, 2048
NH, NKV, HD = 16, 4, 128
GATE_CH = 12
HPC = NH // NKV          # q heads per core
TS = 512                 # token-slice width
NSL = T // TS            # 4 slices
NCK = C // 128           # 16 contraction chunks
TPS = TS // 128          # 4 token tiles per slice
NTT = T // 128           # 16 token tiles
EPS = 1e-6

A_Q = 1.2 / np.sqrt(float(HD))   # rms-norm scale folded into q (incl 1/sqrt(HD))
A_K = 1.2                        # rms-norm scale folded into k
S_Q = float(1.0 / (HD * A_Q * A_Q))
B_Q = float(EPS / (A_Q * A_Q))
S_K = float(1.0 / (HD * A_K * A_K))
B_K = float(EPS / (A_K * A_K))

_compiled = {}


def _ktiles(m4, W):
    """k-tiles overlapping q-slice m4 with their valid tq-column extents.

    Returns list of (n, f0, f1, causal_block_col, edge_block_col); columns are
    relative to the slice (0..TS). First entry covers [0, TS) fully (it opens
    the PSUM accumulation group).
    """
    assert W % 128 == 0 and W >= 384
    out = []
    for n in range(0, TPS * m4 + TPS):
        f0 = max(0, 128 * n - TS * m4)
        f1 = min(TS, 128 * n + W + 128 - TS * m4)
        if f1 <= f0:
            continue
        causal = 128 * n >= TS * m4            # diagonal staircase inside tile
        edge = (128 * n + W + 128 - TS * m4) <= TS  # window lower edge inside
        cb = f0 if causal else None
        eb = (f1 - 128) if edge else None
        out.append((n, f0, f1, cb, eb))
    full = [e for e in out if e[1] == 0 and e[2] == TS]
    assert full, "need one full-extent tile to open the PSUM group"
    first = full[0]
    rest = [e for e in out if e[0] != first[0]]
    return [first] + rest


def _build(W):
    import concourse.bass as bass
    import concourse.tile as tile
    from concourse import bacc, mybir
    from contextlib import ExitStack

    f32 = mybir.dt.float32
    bf16 = mybir.dt.bfloat16
    AF = mybir.ActivationFunctionType

    nc = bacc.Bacc(None, target_bir_lowering=False)

    xd = nc.dram_tensor("xp", [128, NCK, T], bf16, kind="ExternalInput")
    wqd = nc.dram_tensor("wqp", [128, NCK * HPC * HD], bf16, kind="ExternalInput")
    wkd = nc.dram_tensor("wkp", [128, NCK * HD], bf16, kind="ExternalInput")
    wvd = nc.dram_tensor("wvp", [128, NCK * HD], bf16, kind="ExternalInput")
    wpd = nc.dram_tensor("wpp", [128, HPC * C], bf16, kind="ExternalInput")
    ccd = nc.dram_tensor("cc", [128, T], bf16, kind="ExternalInput")
    ssd = nc.dram_tensor("ss", [128, T], bf16, kind="ExternalInput")
    ved = nc.dram_tensor("vep", [128, NTT * HD], bf16, kind="ExternalInput")
    trid = nc.dram_tensor("tri", [128, 256], bf16, kind="ExternalInput")
    outd = nc.dram_tensor("outp", [128, NTT, T], bf16, kind="ExternalOutput")

    with tile.TileContext(nc) as tc, ExitStack() as ctx:
        res = ctx.enter_context(tc.tile_pool(name="res", bufs=1))
        xc_p = ctx.enter_context(tc.tile_pool(name="xc", bufs=2))
        sq_p = ctx.enter_context(tc.tile_pool(name="sq", bufs=2))
        row_p = ctx.enter_context(tc.tile_pool(name="rows", bufs=2))
        bc_p = ctx.enter_context(tc.tile_pool(name="bc", bufs=3))
        qt_p = ctx.enter_context(tc.tile_pool(name="qt", bufs=2))
        es_p = ctx.enter_context(tc.tile_pool(name="es", bufs=4))
        yt_p = ctx.enter_context(tc.tile_pool(name="yt", bufs=2))
        work_p = ctx.enter_context(tc.tile_pool(name="work", bufs=2))
        ot_p = ctx.enter_context(tc.tile_pool(name="ot", bufs=2))

        # PSUM: 8 banks total. qkv(2: k+v concurrently, then cproj co rotation)
        # + s(2: q h0/h2, then the attention S pipeline) + out(2: q h1/h3,
        # then attention ps_out rotation) + aux(1: rms rows at 32h offsets)
        # + sum(1: softmax denominators at 32h offsets).
        ps_qkv = ctx.enter_context(tc.tile_pool(name="ps_qkv", bufs=2, space="PSUM"))
        ps_s = ctx.enter_context(tc.tile_pool(name="ps_s", bufs=2, space="PSUM"))
        ps_out_p = ctx.enter_context(tc.tile_pool(name="ps_out", bufs=2, space="PSUM"))
        ps_aux = ctx.enter_context(tc.tile_pool(name="ps_aux", bufs=1, space="PSUM"))
        ps_sum = ctx.enter_context(tc.tile_pool(name="ps_sum", bufs=1, space="PSUM"))

        # ---- resident tensors ----
        wq_sb = res.tile([128, NCK, HPC * HD], bf16)
        wk_sb = res.tile([128, NCK, HD], bf16)
        wv_sb = res.tile([128, NCK, HD], bf16)
        wp_sb = res.tile([128, HPC, C], bf16)
        cc_sb = res.tile([128, T], bf16)
        ss_sb = res.tile([128, T], bf16)
        ve_sb = res.tile([128, NTT, HD], bf16)
        tri_sb = res.tile([128, 256], bf16)
        kT_sb = res.tile([128, T], bf16)        # rotated+normalized k, HD on partitions
        vn_sb = res.tile([128, NTT, HD], bf16)  # v natural, token tiles on partitions
        ones_sb = res.tile([128, 1], bf16)
        nc.vector.memset(ones_sb, 1.0)
        bq_sb = res.tile([1, 1], f32)
        nc.vector.memset(bq_sb, B_Q)
        bk_sb = res.tile([1, 1], f32)
        nc.vector.memset(bk_sb, B_K)

        xcs = {}
        auxs = {}

        def emit_xload(m4):
            xc = xc_p.tile([128, NCK, TS], bf16, tag="xc")
            t0 = m4 * TS
            for g in range(4):
                nc.sync.dma_start(out=xc[:, 4 * g:4 * g + 4, :],
                                  in_=xd[:, 4 * g:4 * g + 4, t0:t0 + TS])
            xcs[m4] = xc

        def rope(dst, t0):
            """In-place RoPE on a [128, TS] bf16 tile; halves swap via DVE
            reads at a shifted partition base. ss rows 64:128 hold sin, rows
            0:64 hold -sin, matching each op's shared input base."""
            tmp = work_p.tile([128, TS], bf16, tag="tmp")
            nc.vector.tensor_mul(tmp[0:64, :], dst[64:128, :], ss_sb[64:128, t0:t0 + TS])
            nc.vector.tensor_mul(tmp[64:128, :], dst[0:64, :], ss_sb[0:64, t0:t0 + TS])
            nc.vector.tensor_mul(dst, dst, cc_sb[:, t0:t0 + TS])
            nc.vector.tensor_add(dst, dst, tmp)

        def emit_kvproj(m4):
            t0 = m4 * TS
            xc = xcs[m4]
            # k chunk loop, then v in tt-major order: each start=True marks the
            # whole 2048B zero region (= full bank row) pending-zero, so the
            # four v accumulation regions sharing one bank must run strictly
            # one after another — interleaving them clobbers siblings.
            ps_k = ps_qkv.tile([128, TS], f32, tag="qkv")
            for c in range(NCK):
                nc.tensor.matmul(ps_k, wk_sb[:, c, :], xc[:, c, :],
                                 start=(c == 0), stop=(c == NCK - 1))
            ps_v = ps_qkv.tile([128, TPS, HD], f32, tag="qkv")
            for tt in range(TPS):
                for c in range(NCK):
                    nc.tensor.matmul(ps_v[:, tt, :], xc[:, c, tt * 128:(tt + 1) * 128],
                                     wv_sb[:, c, :], start=(c == 0), stop=(c == NCK - 1))
            aux = ps_aux.tile([1, TS], f32, tag="aux")
            auxs[m4] = aux
            sq_k = sq_p.tile([128, TS], bf16, tag="sq")
            nc.scalar.activation(sq_k, ps_k, AF.Square)
            nc.tensor.matmul(aux[0:1, :], ones_sb, sq_k, start=True, stop=True)
            srk = row_p.tile([1, TS], f32, tag="srk")
            nc.scalar.activation(srk, aux[0:1, :], AF.Ln, bias=bk_sb, scale=S_K)
            nc.scalar.activation(srk, srk, AF.Exp, scale=-0.5)
            rbk = bc_p.tile([128, TS], f32, tag="bc")
            nc.gpsimd.partition_broadcast(rbk, srk)
            k_sl = kT_sb[:, t0:t0 + TS]
            nc.vector.tensor_mul(k_sl, ps_k, rbk)
            rope(k_sl, t0)
            for tt in range(TPS):
                nc.vector.tensor_add(vn_sb[:, m4 * TPS + tt, :], ps_v[:, tt, :],
                                     ve_sb[:, m4 * TPS + tt, :])

        def emit_qproj(m4):
            t0 = m4 * TS
            xc = xcs[m4]
            aux = auxs[m4]
            qts = []
            for h in range(HPC):
                pool = ps_s if h % 2 == 0 else ps_out_p
                tag = "s" if h % 2 == 0 else "out"
                ps_q = pool.tile([128, TS], f32, tag=tag)
                for c in range(NCK):
                    nc.tensor.matmul(ps_q, wq_sb[:, c, h * HD:(h + 1) * HD], xc[:, c, :],
                                     start=(c == 0), stop=(c == NCK - 1))
                sq_q = sq_p.tile([128, TS], bf16, tag="sq")
                nc.scalar.activation(sq_q, ps_q, AF.Square)
                # the single aux row is serially reused by k and all q heads:
                # each row is consumed by the Ln activation ~1.2us after it is
                # written, long before the next head's ones-matmul lands.
                nc.tensor.matmul(aux[0:1, :], ones_sb, sq_q, start=True, stop=True)
                srow = row_p.tile([1, TS], f32, tag="srow")
                nc.scalar.activation(srow, aux[0:1, :], AF.Ln,
                                     bias=bq_sb, scale=S_Q)
                nc.scalar.activation(srow, srow, AF.Exp, scale=-0.5)
                rbc = bc_p.tile([128, TS], f32, tag="bc")
                nc.gpsimd.partition_broadcast(rbc, srow)
                qt = qt_p.tile([128, TS], bf16, tag=f"qt{h}")
                nc.vector.tensor_mul(qt, ps_q, rbc)
                rope(qt, t0)
                qts.append(qt)
            return qts

        def make_cproj_fillers(m4, yts):
            """One thunk per c_proj matmul of slice m4 (64 total), in co-major
            order; each accumulates into a rotating ps_qkv bank, evacuates
            at h==3, and DMAs out each finished group of 4 co's."""
            t0 = m4 * TS
            state = {}
            fillers = []
            for co in range(NTT):
                for h in range(HPC):
                    def f(co=co, h=h):
                        if h == 0 and co % 4 == 0:
                            state["ot"] = ot_p.tile([128, 4, TS], bf16, tag="ot",
                                                    name="ot")
                        if h == 0:
                            state["ps"] = ps_qkv.tile([128, TS], f32, tag="qkv",
                                                      name="ps_p")
                        nc.tensor.matmul(state["ps"],
                                         wp_sb[:, h, co * 128:(co + 1) * 128],
                                         yts[h], start=(h == 0), stop=(h == HPC - 1))
                        if h == HPC - 1:
                            nc.vector.tensor_copy(state["ot"][:, co % 4, :], state["ps"])
                            if co % 4 == 3:
                                cg = co // 4
                                nc.sync.dma_start(
                                    out=outd[:, 4 * cg:4 * cg + 4, t0:t0 + TS],
                                    in_=state["ot"])
                    fillers.append(f)
            return fillers

        def emit_attn(m4, qts, fillers):
            tiles = _ktiles(m4, W)
            last = len(tiles) - 1
            nslots = len(tiles) * HPC
            nfill = len(fillers)
            fi = 0
            slot = 0
            # matmul out bases are limited to 0/32/64: heads 0-2 get their own
            # denominator row, head 3 reuses row 0 (h0's recip read happened
            # two head-periods earlier).
            sum4 = ps_sum.tile([65, TS], f32, tag="sum4")
            sum_row = [0, 32, 64, 0]
            yts = []
            for h in range(HPC):
                sr = sum_row[h]
                ps_out = ps_out_p.tile([128, TS], f32, tag="out")
                for idx, (n, f0, f1, cb, eb) in enumerate(tiles):
                    pss = ps_s.tile([128, TS], f32, tag="s")
                    nc.tensor.matmul(pss[:, f0:f1], kT_sb[:, n * 128:(n + 1) * 128],
                                     qts[h][:, f0:f1], start=True, stop=True)
                    # place c_proj filler matmuls right after the S matmul:
                    # the PE chews these while the ACT engine runs exp(i).
                    slot += 1
                    while fi < nfill * slot // nslots:
                        fillers[fi]()
                        fi += 1
                    es = es_p.tile([128, TS], bf16, tag="es")
                    nc.scalar.activation(es[:, f0:f1], pss[:, f0:f1], AF.Exp)
                    if cb is not None:
                        nc.gpsimd.tensor_mul(es[:, cb:cb + 128], es[:, cb:cb + 128],
                                             tri_sb[:, 0:128])
                    if eb is not None:
                        nc.gpsimd.tensor_mul(es[:, eb:eb + 128], es[:, eb:eb + 128],
                                             tri_sb[:, 128:256])
                    nc.tensor.matmul(sum4[sr:sr + 1, f0:f1], ones_sb,
                                     es[:, f0:f1], start=(idx == 0), stop=(idx == last))
                    nc.tensor.matmul(ps_out[:, f0:f1], vn_sb[:, n, :], es[:, f0:f1],
                                     start=(idx == 0), stop=(idx == last))
                rsum = row_p.tile([1, TS], f32, tag="rsum")
                nc.vector.reciprocal(rsum, sum4[sr:sr + 1, :])
                sbc = bc_p.tile([128, TS], f32, tag="bc")
                nc.gpsimd.partition_broadcast(sbc, rsum)
                yt = yt_p.tile([128, TS], bf16, tag=f"yt{h}")
                nc.vector.tensor_mul(yt, ps_out, sbc)
                yts.append(yt)
            while fi < nfill:
                fillers[fi]()
                fi += 1
            return yts

        # ---- prologue: weight/table loads ordered by first use ----
        nc.sync.dma_start(out=wk_sb, in_=wkd[:, :].rearrange("p (c h) -> p c h", c=NCK))
        nc.sync.dma_start(out=wv_sb, in_=wvd[:, :].rearrange("p (c h) -> p c h", c=NCK))
        emit_xload(0)
        nc.sync.dma_start(out=wq_sb, in_=wqd[:, :].rearrange("p (c h) -> p c h", c=NCK))
        nc.sync.dma_start(out=cc_sb, in_=ccd[:, :])
        nc.sync.dma_start(out=ss_sb, in_=ssd[:, :])
        nc.sync.dma_start(out=ve_sb, in_=ved[:, :].rearrange("p (n h) -> p n h", n=NTT))
        nc.sync.dma_start(out=tri_sb, in_=trid[:, :])
        nc.sync.dma_start(out=wp_sb, in_=wpd[:, :].rearrange("p (g o) -> p g o", g=HPC))

        # ---- software-pipelined slice loop ----
        emit_kvproj(0)
        prev = None
        for m4 in range(NSL):
            if m4 + 1 < NSL:
                emit_xload(m4 + 1)
            qts = emit_qproj(m4)
            fillers = make_cproj_fillers(*prev) if prev else []
            yts = emit_attn(m4, qts, fillers)
            if m4 + 1 < NSL:
                emit_kvproj(m4 + 1)
            prev = (m4, yts)
        for f in make_cproj_fillers(*prev):
            f()

    # Restrict the activation-table picker to the one set containing every
    # ACT function we use (exp, ln, square, copy, identity): without this the
    # greedy picker alternates exp_and_others <-> natural_log, inserting a
    # ~1.3us table load per switch. Set ids are positions in act_info.json's
    # list, so unwanted sets are emptied rather than removed.
    import concourse.hw_specs as hw_specs
    import concourse.bacc as bacc_mod

    orig = hw_specs.get_activation_tables

    def only_combined(arch):
        t = orig(arch)
        return {k: (v if k == "natural_log_exp_and_others" else set())
                for k, v in t.items()}

    hw_specs.get_activation_tables = only_combined
    bacc_mod.get_activation_tables = only_combined
    try:
        nc.compile()
    finally:
        hw_specs.get_activation_tables = orig
        bacc_mod.get_activation_tables = orig
    return nc


def _pack_rows(a):
    """[C_rows, F] -> [128, (C_rows/128) * F] SBUF layout (partition-major)."""
    from ml_dtypes import bfloat16

    rows, f = a.shape
    nck = rows // 128
    return np.ascontiguousarray(
        a.reshape(nck, 128, f).transpose(1, 0, 2).reshape(128, nck * f)
    ).astype(bfloat16)


def _prep_inputs(x, ve, cos, sin, Wq, Wk, Wv, Wproj, Wgate, W):
    from ml_dtypes import bfloat16

    cosT = cos[0, :, 0, :].T  # (64, T)
    sinT = sin[0, :, 0, :].T
    cc = np.concatenate([cosT, cosT], axis=0).astype(bfloat16)
    # rows 0:64 = -sin (used by tmp[64:128] = x1 * -sin, both operands base 0),
    # rows 64:128 = sin (used by tmp[0:64] = x2 * sin, both operands base 64).
    ss = np.concatenate([-sinT, sinT], axis=0).astype(bfloat16)
    p = np.arange(128)[:, None]
    f = np.arange(128)[None, :]
    btri = (p <= f).astype(np.float32)
    etri = (f <= p + (W % 128)).astype(np.float32)
    tri = np.concatenate([btri, etri], axis=1).astype(bfloat16)

    xp_by_b = {}
    for b in range(B):
        xp_by_b[b] = _pack_rows(x[b].T)  # [128, NCK*T]

    in_maps = []
    for core in range(8):
        b, g = core // NKV, core % NKV
        hs = slice(g * HPC * HD, (g + 1) * HPC * HD)
        ks = slice(g * HD, (g + 1) * HD)
        gate = 3.0 / (1.0 + np.exp(-(x[b][:, :GATE_CH] @ Wgate[g])))  # (T,)
        vep = gate[:, None] * ve[b][:, ks]  # (T, HD)
        in_maps.append({
            "xp": xp_by_b[b],
            "wqp": _pack_rows(Wq[hs, :].T),
            "wkp": _pack_rows(Wk[ks, :].T),
            "wvp": _pack_rows(Wv[ks, :].T),
            "wpp": _pack_rows(Wproj[:, hs].T),
            "cc": cc,
            "ss": ss,
            "vep": _pack_rows(vep),
            "tri": tri,
        })
    return in_maps


def _run(inputs, trace=False):
    from concourse.bass_utils import run_bass_kernel_spmd

    x = np.asarray(inputs["x"], dtype=np.float32)
    ve = np.asarray(inputs["ve"], dtype=np.float32)
    cos = np.asarray(inputs["cos"], dtype=np.float32)
    sin = np.asarray(inputs["sin"], dtype=np.float32)
    Wq = np.asarray(inputs["Wq"], dtype=np.float32)
    Wk = np.asarray(inputs["Wk"], dtype=np.float32)
    Wv = np.asarray(inputs["Wv"], dtype=np.float32)
    Wproj = np.asarray(inputs["Wproj"], dtype=np.float32)
    Wgate = np.asarray(inputs["Wgate"], dtype=np.float32)
    W = int(inputs["window_size"])

    if W not in _compiled:
        _compiled[W] = _build(W)
    nc = _compiled[W]

    in_maps = _prep_inputs(x, ve, cos, sin, Wq, Wk, Wv, Wproj, Wgate, W)
    res = run_bass_kernel_spmd(nc, in_maps, core_ids=list(range(8)), trace=trace)

    out = np.zeros((B, T, C), dtype=np.float32)
    for core in range(8):
        b = core // NKV
        # outp: [128, NTT, T] with (p, co, t) = partial[co*128+p, t]
        part = np.asarray(res.results[core]["outp"]).astype(np.float32)
        out[b] += part.transpose(1, 0, 2).reshape(C, T).T
    return out, res


def kernel(**inputs):
    out, _ = _run(inputs, trace=False)
    return out


# revision 19
# speedup vs baseline: 1.2706x; 1.0078x over previous
"""Sliding-window causal self-attention (GQA + RoPE + QK-RMSNorm + ve-gate) on
8 Trainium2 NeuronCores.

Sharding: core c handles (batch b = c // 4, kv-head g = c % 4): data parallel
over batch x tensor parallel over the 4 KV head groups (4 query heads per
core). Each core computes its partial c_proj output; the all-reduce over the 4
head shards is a host-side sum.

v3 design (per core):
  - everything the PE touches is bf16 (inputs are host-converted); PSUM
    accumulation stays fp32, so matmul error is input-quantization only.
  - the ve gate (3*sigmoid(x[:,:12] @ Wgate)) is folded into ve on the host:
    ve' = gate * ve, so the device only does v += ve'.
  - k's rms-norm is folded into kT at PSUM evacuation (broadcast row * PSUM),
    so exp() needs no per-key scale and there are no DRAM round trips.
  - v is computed directly in natural (token-partition) layout by using the
    x chunk as the matmul stationary operand: no PE transposes. The k and v
    chunk loops are fused so slice-0 projections track the x DMA arrivals.
  - RoPE's half-swap uses DVE reads at a shifted partition base (the ss table
    is laid out [-sin; sin] so both SBUF inputs of each TensorTensor share a
    partition base, which the BIR verifier requires).
  - scores are computed transposed (S^T: tk x tq); softmax denominators come
    from a ones-stationary matmul into a shared [97, TS] PSUM tile (rows at
    32h: matmul outputs must start at partition 0/32/64/96); no
    max-subtraction (QK rms-norm bounds |score| <= 1.44*sqrt(128)); masking
    multiplies boundary tiles by 0/1 triangles on the Pool engine.
  - c_proj of slice m-1 is interleaved as single-matmul fillers between the
    attention tiles of slice m: the in-order PE would otherwise park at
    sum(i) waiting for exp(i) on the ACT engine (ACT is 2x slower per column
    than the PE).
  - q-head projections alternate between two PSUM pools so the
    square->rownorm->broadcast->evac chain of head h never blocks head h+1.
  - DMA count is ~41 (vs 251 in the original): weights/tables are
    host-prepacked into SBUF layout ([128, free]) so each is one
    large-descriptor DMA; x streams in 4 group-DMAs per 512-token slice;
    output streams out in 4 group-DMAs per slice (bf16 partials, host sums
    in fp32).
"""

import sys

sys.path.insert(0, "/opt/trn_rl_repo")

import numpy as np

B, T, C = 2, 2048, 2048
NH, NKV, HD = 16, 4, 128
GATE_CH = 12
HPC = NH // NKV          # q heads per core
TS = 512                 # token-slice width
NSL = T // TS            # 4 slices
NCK = C // 128           # 16 contraction chunks
TPS = TS // 128          # 4 token tiles per slice
NTT = T // 128           # 16 token tiles
EPS = 1e-6

A_Q = 1.2 / np.sqrt(float(HD))   # rms-norm scale folded into q (incl 1/sqrt(HD))
A_K = 1.2                        # rms-norm scale folded into k
S_Q = float(1.0 / (HD * A_Q * A_Q))
B_Q = float(EPS / (A_Q * A_Q))
S_K = float(1.0 / (HD * A_K * A_K))
B_K = float(EPS / (A_K * A_K))

_compiled = {}


def _ktiles(m4, W):
    """k-tiles overlapping q-slice m4 with their valid tq-column extents.

    Returns list of (n, f0, f1, causal_block_col, edge_block_col); columns are
    relative to the slice (0..TS). First entry covers [0, TS) fully (it opens
    the PSUM accumulation group).
    """
    assert W % 128 == 0 and W >= 384
    out = []
    for n in range(0, TPS * m4 + TPS):
        f0 = max(0, 128 * n - TS * m4)
        f1 = min(TS, 128 * n + W + 128 - TS * m4)
        if f1 <= f0:
            continue
        causal = 128 * n >= TS * m4            # diagonal staircase inside tile
        edge = (128 * n + W + 128 - TS * m4) <= TS  # window lower edge inside
        cb = f0 if causal else None
        eb = (f1 - 128) if edge else None
        out.append((n, f0, f1, cb, eb))
    full = [e for e in out if e[1] == 0 and e[2] == TS]
    assert full, "need one full-extent tile to open the PSUM group"
    first = full[0]
    rest = [e for e in out if e[0] != first[0]]
    return [first] + rest


def _build(W):
    import concourse.bass as bass
    import concourse.tile as tile
    from concourse import bacc, mybir
    from contextlib import ExitStack

    f32 = mybir.dt.float32
    bf16 = mybir.dt.bfloat16
    AF = mybir.ActivationFunctionType

    nc = bacc.Bacc(None, target_bir_lowering=False)

    xd = nc.dram_tensor("xp", [128, NCK, T], bf16, kind="ExternalInput")
    wqd = nc.dram_tensor("wqp", [128, NCK * HPC * HD], bf16, kind="ExternalInput")
    wkd = nc.dram_tensor("wkp", [128, NCK * HD], bf16, kind="ExternalInput")
    wvd = nc.dram_tensor("wvp", [128, NCK * HD], bf16, kind="ExternalInput")
    wpd = nc.dram_tensor("wpp", [128, HPC * C], bf16, kind="ExternalInput")
    ccd = nc.dram_tensor("cc", [128, T], bf16, kind="ExternalInput")
    ssd = nc.dram_tensor("ss", [128, T], bf16, kind="ExternalInput")
    ved = nc.dram_tensor("vep", [128, NTT * HD], bf16, kind="ExternalInput")
    trid = nc.dram_tensor("tri", [128, 256], bf16, kind="ExternalInput")
    outd = nc.dram_tensor("outp", [128, NTT, T], bf16, kind="ExternalOutput")

    with tile.TileContext(nc) as tc, ExitStack() as ctx:
        res = ctx.enter_context(tc.tile_pool(name="res", bufs=1))
        xc_p = ctx.enter_context(tc.tile_pool(name="xc", bufs=2))
        sq_p = ctx.enter_context(tc.tile_pool(name="sq", bufs=2))
        row_p = ctx.enter_context(tc.tile_pool(name="rows", bufs=2))
        bc_p = ctx.enter_context(tc.tile_pool(name="bc", bufs=3))
        qt_p = ctx.enter_context(tc.tile_pool(name="qt", bufs=2))
        es_p = ctx.enter_context(tc.tile_pool(name="es", bufs=4))
        yt_p = ctx.enter_context(tc.tile_pool(name="yt", bufs=2))
        work_p = ctx.enter_context(tc.tile_pool(name="work", bufs=2))
        ot_p = ctx.enter_context(tc.tile_pool(name="ot", bufs=3))

        # PSUM: 8 banks total. qkv(2: k then v, then cproj co rotation)
        # + s(3: q h0/h2, then the attention S pipeline) + out(2: q h1/h3,
        # then attention ps_out rotation) + misc(1: one bank shared serially
        # by the rms row sums (row 0) and the softmax denominators (rows
        # 0/32/64; matmul out bases are limited to 0/32/64)).
        ps_qkv = ctx.enter_context(tc.tile_pool(name="ps_qkv", bufs=2, space="PSUM"))
        ps_s = ctx.enter_context(tc.tile_pool(name="ps_s", bufs=3, space="PSUM"))
        ps_out_p = ctx.enter_context(tc.tile_pool(name="ps_out", bufs=2, space="PSUM"))
        ps_misc = ctx.enter_context(tc.tile_pool(name="ps_misc", bufs=1, space="PSUM"))

        # ---- resident tensors ----
        wq_sb = res.tile([128, NCK, HPC * HD], bf16)
        wk_sb = res.tile([128, NCK, HD], bf16)
        wv_sb = res.tile([128, NCK, HD], bf16)
        wp_sb = res.tile([128, HPC, C], bf16)
        cc_sb = res.tile([128, T], bf16)
        ss_sb = res.tile([128, T], bf16)
        ve_sb = res.tile([128, NTT, HD], bf16)
        tri_sb = res.tile([128, 256], bf16)
        kT_sb = res.tile([128, T], bf16)        # rotated+normalized k, HD on partitions
        vn_sb = res.tile([128, NTT, HD], bf16)  # v natural, token tiles on partitions
        ones_sb = res.tile([128, 1], bf16)
        nc.vector.memset(ones_sb, 1.0)
        bq_sb = res.tile([1, 1], f32)
        nc.vector.memset(bq_sb, B_Q)
        bk_sb = res.tile([1, 1], f32)
        nc.vector.memset(bk_sb, B_K)

        xcs = {}
        auxs = {}

        def emit_xload(m4):
            xc = xc_p.tile([128, NCK, TS], bf16, tag="xc")
            t0 = m4 * TS
            for g in range(4):
                nc.sync.dma_start(out=xc[:, 4 * g:4 * g + 4, :],
                                  in_=xd[:, 4 * g:4 * g + 4, t0:t0 + TS])
            xcs[m4] = xc

        def rope(dst, t0):
            """In-place RoPE on a [128, TS] bf16 tile; halves swap via DVE
            reads at a shifted partition base. ss rows 64:128 hold sin, rows
            0:64 hold -sin, matching each op's shared input base."""
            tmp = work_p.tile([128, TS], bf16, tag="tmp")
            nc.vector.tensor_mul(tmp[0:64, :], dst[64:128, :], ss_sb[64:128, t0:t0 + TS])
            nc.vector.tensor_mul(tmp[64:128, :], dst[0:64, :], ss_sb[0:64, t0:t0 + TS])
            nc.vector.tensor_mul(dst, dst, cc_sb[:, t0:t0 + TS])
            nc.vector.tensor_add(dst, dst, tmp)

        def emit_kvproj(m4):
            t0 = m4 * TS
            xc = xcs[m4]
            # k chunk loop, then v in tt-major order: each start=True marks the
            # whole 2048B zero region (= full bank row) pending-zero, so the
            # four v accumulation regions sharing one bank must run strictly
            # one after another — interleaving them clobbers siblings.
            ps_k = ps_qkv.tile([128, TS], f32, tag="qkv")
            for c in range(NCK):
                nc.tensor.matmul(ps_k, wk_sb[:, c, :], xc[:, c, :],
                                 start=(c == 0), stop=(c == NCK - 1))
            ps_v = ps_qkv.tile([128, TPS, HD], f32, tag="qkv")
            for tt in range(TPS):
                for c in range(NCK):
                    nc.tensor.matmul(ps_v[:, tt, :], xc[:, c, tt * 128:(tt + 1) * 128],
                                     wv_sb[:, c, :], start=(c == 0), stop=(c == NCK - 1))
            aux = ps_misc.tile([65, TS], f32, tag="misc")
            auxs[m4] = aux
            sq_k = sq_p.tile([128, TS], bf16, tag="sq")
            nc.scalar.activation(sq_k, ps_k, AF.Square)
            nc.tensor.matmul(aux[0:1, :], ones_sb, sq_k, start=True, stop=True)
            srk = row_p.tile([1, TS], f32, tag="srk")
            nc.scalar.activation(srk, aux[0:1, :], AF.Ln, bias=bk_sb, scale=S_K)
            nc.scalar.activation(srk, srk, AF.Exp, scale=-0.5)
            rbk = bc_p.tile([128, TS], f32, tag="bc")
            nc.gpsimd.partition_broadcast(rbk, srk)
            k_sl = kT_sb[:, t0:t0 + TS]
            nc.vector.tensor_mul(k_sl, ps_k, rbk)
            rope(k_sl, t0)
            for tt in range(TPS):
                nc.vector.tensor_add(vn_sb[:, m4 * TPS + tt, :], ps_v[:, tt, :],
                                     ve_sb[:, m4 * TPS + tt, :])

        def emit_qproj(m4):
            t0 = m4 * TS
            xc = xcs[m4]
            aux = auxs[m4]
            qts = []
            for h in range(HPC):
                pool = ps_s if h % 2 == 0 else ps_out_p
                tag = "s" if h % 2 == 0 else "out"
                ps_q = pool.tile([128, TS], f32, tag=tag)
                for c in range(NCK):
                    nc.tensor.matmul(ps_q, wq_sb[:, c, h * HD:(h + 1) * HD], xc[:, c, :],
                                     start=(c == 0), stop=(c == NCK - 1))
                sq_q = sq_p.tile([128, TS], bf16, tag="sq")
                nc.scalar.activation(sq_q, ps_q, AF.Square)
                # the single aux row is serially reused by k and all q heads:
                # each row is consumed by the Ln activation ~1.2us after it is
                # written, long before the next head's ones-matmul lands.
                nc.tensor.matmul(aux[0:1, :], ones_sb, sq_q, start=True, stop=True)
                srow = row_p.tile([1, TS], f32, tag="srow")
                nc.scalar.activation(srow, aux[0:1, :], AF.Ln,
                                     bias=bq_sb, scale=S_Q)
                nc.scalar.activation(srow, srow, AF.Exp, scale=-0.5)
                rbc = bc_p.tile([128, TS], f32, tag="bc")
                nc.gpsimd.partition_broadcast(rbc, srow)
                qt = qt_p.tile([128, TS], bf16, tag=f"qt{h}")
                nc.vector.tensor_mul(qt, ps_q, rbc)
                rope(qt, t0)
                qts.append(qt)
            return qts

        def make_cproj_fillers(m4, yts, tail=False):
            """One thunk per c_proj matmul of slice m4 (64 total), in co-major
            order; each accumulates into a rotating ps_qkv bank, evacuates
            at h==3, and DMAs out each finished group of 4 co's. For the
            kernel tail the last group streams out per-co so the final DMA
            only carries 128KB."""
            t0 = m4 * TS
            state = {}
            fillers = []
            for co in range(NTT):
                for h in range(HPC):
                    def f(co=co, h=h):
                        percol = tail and co >= NTT - 4
                        if h == 0 and co % 4 == 0:
                            state["ot"] = ot_p.tile([128, 4, TS], bf16, tag="ot",
                                                    name="ot")
                        if h == 0:
                            state["ps"] = ps_qkv.tile([128, TS], f32, tag="qkv",
                                                      name="ps_p")
                        nc.tensor.matmul(state["ps"],
                                         wp_sb[:, h, co * 128:(co + 1) * 128],
                                         yts[h], start=(h == 0), stop=(h == HPC - 1))
                        if h == HPC - 1:
                            nc.vector.tensor_copy(state["ot"][:, co % 4, :], state["ps"])
                            if percol:
                                nc.sync.dma_start(
                                    out=outd[:, co:co + 1, t0:t0 + TS],
                                    in_=state["ot"][:, co % 4:co % 4 + 1, :])
                            elif co % 4 == 3:
                                cg = co // 4
                                nc.sync.dma_start(
                                    out=outd[:, 4 * cg:4 * cg + 4, t0:t0 + TS],
                                    in_=state["ot"])
                    fillers.append(f)
            return fillers

        def emit_attn(m4, qts, fillers):
            tiles = _ktiles(m4, W)
            last = len(tiles) - 1
            nslots = len(tiles) * HPC
            nfill = len(fillers)
            fi = 0
            slot = 0
            # denominators live in the same bank as the rms rows: heads 0-2
            # get rows 0/32/64, head 3 reuses row 0 (h0's recip read happened
            # two head-periods earlier).
            sum4 = auxs[m4]
            sum_row = [0, 32, 64, 0]
            yts = []
            for h in range(HPC):
                sr = sum_row[h]
                ps_out = ps_out_p.tile([128, TS], f32, tag="out")
                for idx, (n, f0, f1, cb, eb) in enumerate(tiles):
                    pss = ps_s.tile([128, TS], f32, tag="s")
                    nc.tensor.matmul(pss[:, f0:f1], kT_sb[:, n * 128:(n + 1) * 128],
                                     qts[h][:, f0:f1], start=True, stop=True)
                    # place c_proj filler matmuls right after the S matmul:
                    # the PE chews these while the ACT engine runs exp(i).
                    # Front-load two extra fillers on the first tiles of each
                    # head — the exp pipeline needs runway there.
                    slot += 1
                    target = nfill * slot // nslots + (2 if idx < 2 else 0)
                    while fi < min(nfill, target):
                        fillers[fi]()
                        fi += 1
                    es = es_p.tile([128, TS], bf16, tag="es")
                    nc.scalar.activation(es[:, f0:f1], pss[:, f0:f1], AF.Exp)
                    if cb is not None:
                        nc.gpsimd.tensor_mul(es[:, cb:cb + 128], es[:, cb:cb + 128],
                                             tri_sb[:, 0:128])
                    if eb is not None:
                        nc.gpsimd.tensor_mul(es[:, eb:eb + 128], es[:, eb:eb + 128],
                                             tri_sb[:, 128:256])
                    nc.tensor.matmul(sum4[sr:sr + 1, f0:f1], ones_sb,
                                     es[:, f0:f1], start=(idx == 0), stop=(idx == last))
                    nc.tensor.matmul(ps_out[:, f0:f1], vn_sb[:, n, :], es[:, f0:f1],
                                     start=(idx == 0), stop=(idx == last))
                rsum = row_p.tile([1, TS], f32, tag="rsum")
                nc.vector.reciprocal(rsum, sum4[sr:sr + 1, :])
                sbc = bc_p.tile([128, TS], f32, tag="bc")
                nc.gpsimd.partition_broadcast(sbc, rsum)
                yt = yt_p.tile([128, TS], bf16, tag=f"yt{h}")
                nc.vector.tensor_mul(yt, ps_out, sbc)
                yts.append(yt)
            while fi < nfill:
                fillers[fi]()
                fi += 1
            return yts

        # ---- prologue: weight/table loads ordered by first use ----
        nc.sync.dma_start(out=wk_sb, in_=wkd[:, :].rearrange("p (c h) -> p c h", c=NCK))
        nc.sync.dma_start(out=wv_sb, in_=wvd[:, :].rearrange("p (c h) -> p c h", c=NCK))
        emit_xload(0)
        nc.sync.dma_start(out=wq_sb, in_=wqd[:, :].rearrange("p (c h) -> p c h", c=NCK))
        nc.sync.dma_start(out=cc_sb, in_=ccd[:, :])
        nc.sync.dma_start(out=ss_sb, in_=ssd[:, :])
        nc.sync.dma_start(out=ve_sb, in_=ved[:, :].rearrange("p (n h) -> p n h", n=NTT))
        nc.sync.dma_start(out=tri_sb, in_=trid[:, :])
        nc.sync.dma_start(out=wp_sb, in_=wpd[:, :].rearrange("p (g o) -> p g o", g=HPC))

        # ---- software-pipelined slice loop ----
        emit_kvproj(0)
        prev = None
        for m4 in range(NSL):
            if m4 + 1 < NSL:
                emit_xload(m4 + 1)
            qts = emit_qproj(m4)
            fillers = make_cproj_fillers(*prev) if prev else []
            yts = emit_attn(m4, qts, fillers)
            if m4 + 1 < NSL:
                emit_kvproj(m4 + 1)
            prev = (m4, yts)
        for f in make_cproj_fillers(*prev, tail=True):
            f()

    # Restrict the activation-table picker to the one set containing every
    # ACT function we use (exp, ln, square, copy, identity): without this the
    # greedy picker alternates exp_and_others <-> natural_log, inserting a
    # ~1.3us table load per switch. Set ids are positions in act_info.json's
    # list, so unwanted sets are emptied rather than removed.
    import concourse.hw_specs as hw_specs
    import concourse.bacc as bacc_mod

    orig = hw_specs.get_activation_tables

    def only_combined(arch):
        t = orig(arch)
        return {k: (v if k == "natural_log_exp_and_others" else set())
                for k, v in t.items()}

    hw_specs.get_activation_tables = only_combined
    bacc_mod.get_activation_tables = only_combined
    try:
        nc.compile()
    finally:
        hw_specs.get_activation_tables = orig
        bacc_mod.get_activation_tables = orig
    return nc


def _pack_rows(a):
    """[C_rows, F] -> [128, (C_rows/128) * F] SBUF layout (partition-major)."""
    from ml_dtypes import bfloat16

    rows, f = a.shape
    nck = rows // 128
    return np.ascontiguousarray(
        a.reshape(nck, 128, f).transpose(1, 0, 2).reshape(128, nck * f)
    ).astype(bfloat16)


def _prep_inputs(x, ve, cos, sin, Wq, Wk, Wv, Wproj, Wgate, W):
    from ml_dtypes import bfloat16

    cosT = cos[0, :, 0, :].T  # (64, T)
    sinT = sin[0, :, 0, :].T
    cc = np.concatenate([cosT, cosT], axis=0).astype(bfloat16)
    # rows 0:64 = -sin (used by tmp[64:128] = x1 * -sin, both operands base 0),
    # rows 64:128 = sin (used by tmp[0:64] = x2 * sin, both operands base 64).
    ss = np.concatenate([-sinT, sinT], axis=0).astype(bfloat16)
    p = np.arange(128)[:, None]
    f = np.arange(128)[None, :]
    btri = (p <= f).astype(np.float32)
    etri = (f <= p + (W % 128)).astype(np.float32)
    tri = np.concatenate([btri, etri], axis=1).astype(bfloat16)

    xp_by_b = {}
    for b in range(B):
        xp_by_b[b] = _pack_rows(x[b].T)  # [128, NCK*T]

    in_maps = []
    for core in range(8):
        b, g = core // NKV, core % NKV
        hs = slice(g * HPC * HD, (g + 1) * HPC * HD)
        ks = slice(g * HD, (g + 1) * HD)
        gate = 3.0 / (1.0 + np.exp(-(x[b][:, :GATE_CH] @ Wgate[g])))  # (T,)
        vep = gate[:, None] * ve[b][:, ks]  # (T, HD)
        in_maps.append({
            "xp": xp_by_b[b],
            "wqp": _pack_rows(Wq[hs, :].T),
            "wkp": _pack_rows(Wk[ks, :].T),
            "wvp": _pack_rows(Wv[ks, :].T),
            "wpp": _pack_rows(Wproj[:, hs].T),
            "cc": cc,
            "ss": ss,
            "vep": _pack_rows(vep),
            "tri": tri,
        })
    return in_maps


def _run(inputs, trace=False):
    from concourse.bass_utils import run_bass_kernel_spmd

    x = np.asarray(inputs["x"], dtype=np.float32)
    ve = np.asarray(inputs["ve"], dtype=np.float32)
    cos = np.asarray(inputs["cos"], dtype=np.float32)
    sin = np.asarray(inputs["sin"], dtype=np.float32)
    Wq = np.asarray(inputs["Wq"], dtype=np.float32)
    Wk = np.asarray(inputs["Wk"], dtype=np.float32)
    Wv = np.asarray(inputs["Wv"], dtype=np.float32)
    Wproj = np.asarray(inputs["Wproj"], dtype=np.float32)
    Wgate = np.asarray(inputs["Wgate"], dtype=np.float32)
    W = int(inputs["window_size"])

    if W not in _compiled:
        _compiled[W] = _build(W)
    nc = _compiled[W]

    in_maps = _prep_inputs(x, ve, cos, sin, Wq, Wk, Wv, Wproj, Wgate, W)
    res = run_bass_kernel_spmd(nc, in_maps, core_ids=list(range(8)), trace=trace)

    out = np.zeros((B, T, C), dtype=np.float32)
    for core in range(8):
        b = core // NKV
        # outp: [128, NTT, T] with (p, co, t) = partial[co*128+p, t]
        part = np.asarray(res.results[core]["outp"]).astype(np.float32)
        out[b] += part.transpose(1, 0, 2).reshape(C, T).T
    return out, res


def kernel(**inputs):
    out, _ = _run(inputs, trace=False)
    return out


# revision 24
# speedup vs baseline: 1.3024x; 1.0250x over previous
"""Sliding-window causal self-attention (GQA + RoPE + QK-RMSNorm + ve-gate) on
8 Trainium2 NeuronCores.

Sharding: core c handles (batch b = c // 4, kv-head g = c % 4): data parallel
over batch x tensor parallel over the 4 KV head groups (4 query heads per
core). Each core computes its partial c_proj output; the all-reduce over the 4
head shards is a host-side sum.

v3 design (per core):
  - everything the PE touches is bf16 (inputs are host-converted); PSUM
    accumulation stays fp32, so matmul error is input-quantization only.
  - the ve gate (3*sigmoid(x[:,:12] @ Wgate)) is folded into ve on the host:
    ve' = gate * ve, so the device only does v += ve'.
  - k's rms-norm is folded into kT at PSUM evacuation (broadcast row * PSUM),
    so exp() needs no per-key scale and there are no DRAM round trips.
  - v is computed directly in natural (token-partition) layout by using the
    x chunk as the matmul stationary operand: no PE transposes. The k and v
    chunk loops are fused so slice-0 projections track the x DMA arrivals.
  - RoPE's half-swap uses DVE reads at a shifted partition base (the ss table
    is laid out [-sin; sin] so both SBUF inputs of each TensorTensor share a
    partition base, which the BIR verifier requires).
  - scores are computed transposed (S^T: tk x tq); softmax denominators come
    from a ones-stationary matmul into a shared [97, TS] PSUM tile (rows at
    32h: matmul outputs must start at partition 0/32/64/96); no
    max-subtraction (QK rms-norm bounds |score| <= 1.44*sqrt(128)); masking
    multiplies boundary tiles by 0/1 triangles on the Pool engine.
  - c_proj of slice m-1 is interleaved as single-matmul fillers between the
    attention tiles of slice m: the in-order PE would otherwise park at
    sum(i) waiting for exp(i) on the ACT engine (ACT is 2x slower per column
    than the PE).
  - q-head projections alternate between two PSUM pools so the
    square->rownorm->broadcast->evac chain of head h never blocks head h+1.
  - DMA count is ~41 (vs 251 in the original): weights/tables are
    host-prepacked into SBUF layout ([128, free]) so each is one
    large-descriptor DMA; x streams in 4 group-DMAs per 512-token slice;
    output streams out in 4 group-DMAs per slice (bf16 partials, host sums
    in fp32).
"""

import sys

sys.path.insert(0, "/opt/trn_rl_repo")

import numpy as np

B, T, C = 2, 2048, 2048
NH, NKV, HD = 16, 4, 128
GATE_CH = 12
HPC = NH // NKV          # q heads per core
TS = 512                 # token-slice width
NSL = T // TS            # 4 slices
NCK = C // 128           # 16 contraction chunks
TPS = TS // 128          # 4 token tiles per slice
NTT = T // 128           # 16 token tiles
EPS = 1e-6

A_Q = 1.2 / np.sqrt(float(HD))   # rms-norm scale folded into q (incl 1/sqrt(HD))
A_K = 1.2                        # rms-norm scale folded into k
S_Q = float(1.0 / (HD * A_Q * A_Q))
B_Q = float(EPS / (A_Q * A_Q))
S_K = float(1.0 / (HD * A_K * A_K))
B_K = float(EPS / (A_K * A_K))

_compiled = {}


def _ktiles(m4, W):
    """k-tiles overlapping q-slice m4 with their valid tq-column extents.

    Returns list of (n, f0, f1, causal_block_col, edge_block_col); columns are
    relative to the slice (0..TS). First entry covers [0, TS) fully (it opens
    the PSUM accumulation group).
    """
    assert W % 128 == 0 and W >= 384
    out = []
    for n in range(0, TPS * m4 + TPS):
        f0 = max(0, 128 * n - TS * m4)
        f1 = min(TS, 128 * n + W + 128 - TS * m4)
        if f1 <= f0:
            continue
        causal = 128 * n >= TS * m4            # diagonal staircase inside tile
        edge = (128 * n + W + 128 - TS * m4) <= TS  # window lower edge inside
        cb = f0 if causal else None
        eb = (f1 - 128) if edge else None
        out.append((n, f0, f1, cb, eb))
    full = [e for e in out if e[1] == 0 and e[2] == TS]
    assert full, "need one full-extent tile to open the PSUM group"
    first = full[0]
    rest = [e for e in out if e[0] != first[0]]
    return [first] + rest


def _build(W):
    import concourse.bass as bass
    import concourse.tile as tile
    from concourse import bacc, mybir
    from contextlib import ExitStack

    f32 = mybir.dt.float32
    bf16 = mybir.dt.bfloat16
    AF = mybir.ActivationFunctionType

    nc = bacc.Bacc(None, target_bir_lowering=False)

    xd = nc.dram_tensor("xp", [128, NCK, T], bf16, kind="ExternalInput")
    wqd = nc.dram_tensor("wqp", [128, NCK * HPC * HD], bf16, kind="ExternalInput")
    wkd = nc.dram_tensor("wkp", [128, NCK * HD], bf16, kind="ExternalInput")
    wvd = nc.dram_tensor("wvp", [128, NCK * HD], bf16, kind="ExternalInput")
    wpd = nc.dram_tensor("wpp", [128, HPC * C], bf16, kind="ExternalInput")
    ccd = nc.dram_tensor("cc", [128, T], bf16, kind="ExternalInput")
    ssd = nc.dram_tensor("ss", [128, T], bf16, kind="ExternalInput")
    ved = nc.dram_tensor("vep", [128, NTT * HD], bf16, kind="ExternalInput")
    trid = nc.dram_tensor("tri", [128, 384], bf16, kind="ExternalInput")
    outd = nc.dram_tensor("outp", [128, NTT, T], bf16, kind="ExternalOutput")

    with tile.TileContext(nc) as tc, ExitStack() as ctx:
        res = ctx.enter_context(tc.tile_pool(name="res", bufs=1))
        xc_p = ctx.enter_context(tc.tile_pool(name="xc", bufs=2))
        sq_p = ctx.enter_context(tc.tile_pool(name="sq", bufs=2))
        row_p = ctx.enter_context(tc.tile_pool(name="rows", bufs=2))
        bc_p = ctx.enter_context(tc.tile_pool(name="bc", bufs=3))
        qt_p = ctx.enter_context(tc.tile_pool(name="qt", bufs=2))
        es_p = ctx.enter_context(tc.tile_pool(name="es", bufs=4))
        yt_p = ctx.enter_context(tc.tile_pool(name="yt", bufs=2))
        work_p = ctx.enter_context(tc.tile_pool(name="work", bufs=2))
        ot_p = ctx.enter_context(tc.tile_pool(name="ot", bufs=3))

        # PSUM: 8 banks total. qkv(2: k then v, then cproj co rotation)
        # + s(3: q h0/h2, then the attention S pipeline) + out(2: q h1/h3,
        # then attention ps_out rotation) + misc(1: one bank shared serially
        # by the rms row sums (row 0) and the softmax denominators (rows
        # 0/32/64; matmul out bases are limited to 0/32/64)).
        ps_qkv = ctx.enter_context(tc.tile_pool(name="ps_qkv", bufs=2, space="PSUM"))
        ps_s = ctx.enter_context(tc.tile_pool(name="ps_s", bufs=3, space="PSUM"))
        ps_out_p = ctx.enter_context(tc.tile_pool(name="ps_out", bufs=2, space="PSUM"))
        ps_misc = ctx.enter_context(tc.tile_pool(name="ps_misc", bufs=1, space="PSUM"))

        # ---- resident tensors ----
        wq_sb = res.tile([128, NCK, HPC * HD], bf16)
        wk_sb = res.tile([128, NCK, HD], bf16)
        wv_sb = res.tile([128, NCK, HD], bf16)
        wp_sb = res.tile([128, HPC, C], bf16)
        cc_sb = res.tile([128, T], bf16)
        ss_sb = res.tile([128, T], bf16)
        ve_sb = res.tile([128, NTT, HD], bf16)
        tri_sb = res.tile([128, 384], bf16)  # [Mc^T | Me^T | identity]
        kT_sb = res.tile([128, T], bf16)        # rotated+normalized k, HD on partitions
        vn_sb = res.tile([128, NTT, HD], bf16)  # v natural, token tiles on partitions
        ones_sb = res.tile([128, 1], bf16)
        nc.vector.memset(ones_sb, 1.0)
        bq_sb = res.tile([1, 1], f32)
        nc.vector.memset(bq_sb, B_Q)
        bk_sb = res.tile([1, 1], f32)
        nc.vector.memset(bk_sb, B_K)

        xcs = {}
        auxs = {}

        def emit_xload(m4):
            xc = xc_p.tile([128, NCK, TS], bf16, tag="xc")
            t0 = m4 * TS
            for g in range(4):
                nc.sync.dma_start(out=xc[:, 4 * g:4 * g + 4, :],
                                  in_=xd[:, 4 * g:4 * g + 4, t0:t0 + TS])
            xcs[m4] = xc

        def rope(dst, t0):
            """In-place RoPE on a [128, TS] bf16 tile; halves swap via DVE
            reads at a shifted partition base. ss rows 64:128 hold sin, rows
            0:64 hold -sin, matching each op's shared input base."""
            tmp = work_p.tile([128, TS], bf16, tag="tmp")
            nc.vector.tensor_mul(tmp[0:64, :], dst[64:128, :], ss_sb[64:128, t0:t0 + TS])
            nc.vector.tensor_mul(tmp[64:128, :], dst[0:64, :], ss_sb[0:64, t0:t0 + TS])
            nc.vector.tensor_mul(dst, dst, cc_sb[:, t0:t0 + TS])
            nc.vector.tensor_add(dst, dst, tmp)

        def emit_kvproj(m4):
            t0 = m4 * TS
            xc = xcs[m4]
            # k chunk loop, then v in tt-major order: each start=True marks the
            # whole 2048B zero region (= full bank row) pending-zero, so the
            # four v accumulation regions sharing one bank must run strictly
            # one after another — interleaving them clobbers siblings.
            ps_k = ps_qkv.tile([128, TS], f32, tag="qkv")
            for c in range(NCK):
                nc.tensor.matmul(ps_k, wk_sb[:, c, :], xc[:, c, :],
                                 start=(c == 0), stop=(c == NCK - 1))
            ps_v = ps_qkv.tile([128, TPS, HD], f32, tag="qkv")
            for tt in range(TPS):
                for c in range(NCK):
                    nc.tensor.matmul(ps_v[:, tt, :], xc[:, c, tt * 128:(tt + 1) * 128],
                                     wv_sb[:, c, :], start=(c == 0), stop=(c == NCK - 1))
            aux = ps_misc.tile([65, TS], f32, tag="misc")
            auxs[m4] = aux
            sq_k = sq_p.tile([128, TS], bf16, tag="sq")
            nc.scalar.activation(sq_k, ps_k, AF.Square)
            nc.tensor.matmul(aux[0:1, :], ones_sb, sq_k, start=True, stop=True)
            srk = row_p.tile([1, TS], f32, tag="srk")
            nc.scalar.activation(srk, aux[0:1, :], AF.Ln, bias=bk_sb, scale=S_K)
            nc.scalar.activation(srk, srk, AF.Exp, scale=-0.5)
            rbk = bc_p.tile([128, TS], f32, tag="bc")
            nc.gpsimd.partition_broadcast(rbk, srk)
            k_sl = kT_sb[:, t0:t0 + TS]
            nc.vector.tensor_mul(k_sl, ps_k, rbk)
            rope(k_sl, t0)
            for tt in range(TPS):
                nc.vector.tensor_add(vn_sb[:, m4 * TPS + tt, :], ps_v[:, tt, :],
                                     ve_sb[:, m4 * TPS + tt, :])

        def emit_qproj(m4):
            t0 = m4 * TS
            xc = xcs[m4]
            aux = auxs[m4]
            qts = []
            for h in range(HPC):
                pool = ps_s if h % 2 == 0 else ps_out_p
                tag = "s" if h % 2 == 0 else "out"
                ps_q = pool.tile([128, TS], f32, tag=tag)
                for c in range(NCK):
                    nc.tensor.matmul(ps_q, wq_sb[:, c, h * HD:(h + 1) * HD], xc[:, c, :],
                                     start=(c == 0), stop=(c == NCK - 1))
                sq_q = sq_p.tile([128, TS], bf16, tag="sq")
                nc.scalar.activation(sq_q, ps_q, AF.Square)
                # the single aux row is serially reused by k and all q heads:
                # each row is consumed by the Ln activation ~1.2us after it is
                # written, long before the next head's ones-matmul lands.
                nc.tensor.matmul(aux[0:1, :], ones_sb, sq_q, start=True, stop=True)
                srow = row_p.tile([1, TS], f32, tag="srow")
                nc.scalar.activation(srow, aux[0:1, :], AF.Ln,
                                     bias=bq_sb, scale=S_Q)
                nc.scalar.activation(srow, srow, AF.Exp, scale=-0.5)
                rbc = bc_p.tile([128, TS], f32, tag="bc")
                nc.gpsimd.partition_broadcast(rbc, srow)
                qt = qt_p.tile([128, TS], bf16, tag=f"qt{h}")
                nc.vector.tensor_mul(qt, ps_q, rbc)
                rope(qt, t0)
                qts.append(qt)
            return qts

        def make_cproj_fillers(m4, yts, tail=False):
            """One thunk per c_proj matmul of slice m4 (64 total), in co-major
            order; each accumulates into a rotating ps_qkv bank, evacuates
            at h==3, and DMAs out each finished group of 4 co's. For the
            kernel tail the last group streams out per-co so the final DMA
            only carries 128KB."""
            t0 = m4 * TS
            state = {}
            fillers = []
            for co in range(NTT):
                for h in range(HPC):
                    def f(co=co, h=h):
                        percol = tail and co >= NTT - 4
                        if h == 0 and co % 4 == 0:
                            state["ot"] = ot_p.tile([128, 4, TS], bf16, tag="ot",
                                                    name="ot")
                        if h == 0:
                            state["ps"] = ps_qkv.tile([128, TS], f32, tag="qkv",
                                                      name="ps_p")
                        nc.tensor.matmul(state["ps"],
                                         wp_sb[:, h, co * 128:(co + 1) * 128],
                                         yts[h], start=(h == 0), stop=(h == HPC - 1))
                        if h == HPC - 1:
                            nc.vector.tensor_copy(state["ot"][:, co % 4, :], state["ps"])
                            if percol:
                                nc.sync.dma_start(
                                    out=outd[:, co:co + 1, t0:t0 + TS],
                                    in_=state["ot"][:, co % 4:co % 4 + 1, :])
                            elif co % 4 == 3:
                                cg = co // 4
                                nc.sync.dma_start(
                                    out=outd[:, 4 * cg:4 * cg + 4, t0:t0 + TS],
                                    in_=state["ot"])
                    fillers.append(f)
            return fillers

        def emit_attn(m4, qts, fillers):
            tiles = _ktiles(m4, W)
            last = len(tiles) - 1
            wts = [3 if i == 0 else (2 if i == 1 else 1) for i in range(len(tiles))]
            wsum = sum(wts) * HPC
            nfill = len(fillers)
            fi = 0
            slot = 0
            # denominators live in the same bank as the rms rows: heads 0-2
            # get rows 0/32/64, head 3 reuses row 0 (h0's recip read happened
            # two head-periods earlier).
            sum4 = auxs[m4]
            sum_row = [0, 32, 64, 0]
            yts = []
            for h in range(HPC):
                sr = sum_row[h]
                ps_out = ps_out_p.tile([128, TS], f32, tag="out")
                for idx, (n, f0, f1, cb, eb) in enumerate(tiles):
                    pss = ps_s.tile([128, TS], f32, tag="s")
                    masked = (cb is not None) + (eb is not None)
                    nc.tensor.matmul(pss[:, f0:f1], kT_sb[:, n * 128:(n + 1) * 128],
                                     qts[h][:, f0:f1], start=True, stop=(masked == 0))
                    # boundary masking as a pre-exp -100 additive matmul
                    # (mask^T stationary, identity moving) in the same PSUM
                    # group: exp then yields exact zeros and nothing waits on
                    # the Pool engine.
                    if cb is not None:
                        masked -= 1
                        nc.tensor.matmul(pss[:, cb:cb + 128], tri_sb[:, 0:128],
                                         tri_sb[:, 256:384], start=False,
                                         stop=(masked == 0))
                    if eb is not None:
                        masked -= 1
                        nc.tensor.matmul(pss[:, eb:eb + 128], tri_sb[:, 128:256],
                                         tri_sb[:, 256:384], start=False,
                                         stop=(masked == 0))
                    # place c_proj filler matmuls right after the S matmul:
                    # the PE chews these while the ACT engine runs exp(i).
                    # Pacing is weighted 3/2/1 toward the first tiles of each
                    # head — the exp pipeline needs runway there.
                    slot += wts[idx]
                    while fi < min(nfill, nfill * slot // wsum):
                        fillers[fi]()
                        fi += 1
                    es = es_p.tile([128, TS], bf16, tag="es")
                    nc.scalar.activation(es[:, f0:f1], pss[:, f0:f1], AF.Exp)
                    nc.tensor.matmul(sum4[sr:sr + 1, f0:f1], ones_sb,
                                     es[:, f0:f1], start=(idx == 0), stop=(idx == last))
                    nc.tensor.matmul(ps_out[:, f0:f1], vn_sb[:, n, :], es[:, f0:f1],
                                     start=(idx == 0), stop=(idx == last))
                rsum = row_p.tile([1, TS], f32, tag="rsum")
                nc.vector.reciprocal(rsum, sum4[sr:sr + 1, :])
                sbc = bc_p.tile([128, TS], f32, tag="bc")
                nc.gpsimd.partition_broadcast(sbc, rsum)
                yt = yt_p.tile([128, TS], bf16, tag=f"yt{h}")
                nc.vector.tensor_mul(yt, ps_out, sbc)
                yts.append(yt)
            while fi < nfill:
                fillers[fi]()
                fi += 1
            return yts

        # ---- prologue: weight/table loads ordered by first use ----
        nc.sync.dma_start(out=wk_sb, in_=wkd[:, :].rearrange("p (c h) -> p c h", c=NCK))
        nc.sync.dma_start(out=wv_sb, in_=wvd[:, :].rearrange("p (c h) -> p c h", c=NCK))
        emit_xload(0)
        nc.sync.dma_start(out=wq_sb, in_=wqd[:, :].rearrange("p (c h) -> p c h", c=NCK))
        nc.sync.dma_start(out=cc_sb, in_=ccd[:, :])
        nc.sync.dma_start(out=ss_sb, in_=ssd[:, :])
        nc.sync.dma_start(out=ve_sb, in_=ved[:, :].rearrange("p (n h) -> p n h", n=NTT))
        nc.sync.dma_start(out=tri_sb, in_=trid[:, :])
        nc.sync.dma_start(out=wp_sb, in_=wpd[:, :].rearrange("p (g o) -> p g o", g=HPC))

        # ---- software-pipelined slice loop ----
        emit_kvproj(0)
        prev = None
        for m4 in range(NSL):
            if m4 + 1 < NSL:
                emit_xload(m4 + 1)
            qts = emit_qproj(m4)
            fillers = make_cproj_fillers(*prev) if prev else []
            yts = emit_attn(m4, qts, fillers)
            if m4 + 1 < NSL:
                emit_kvproj(m4 + 1)
            prev = (m4, yts)
        for f in make_cproj_fillers(*prev, tail=True):
            f()

    # Restrict the activation-table picker to the one set containing every
    # ACT function we use (exp, ln, square, copy, identity): without this the
    # greedy picker alternates exp_and_others <-> natural_log, inserting a
    # ~1.3us table load per switch. Set ids are positions in act_info.json's
    # list, so unwanted sets are emptied rather than removed.
    import concourse.hw_specs as hw_specs
    import concourse.bacc as bacc_mod

    orig = hw_specs.get_activation_tables

    def only_combined(arch):
        t = orig(arch)
        return {k: (v if k == "natural_log_exp_and_others" else set())
                for k, v in t.items()}

    hw_specs.get_activation_tables = only_combined
    bacc_mod.get_activation_tables = only_combined
    try:
        nc.compile()
    finally:
        hw_specs.get_activation_tables = orig
        bacc_mod.get_activation_tables = orig
    return nc


def _pack_rows(a):
    """[C_rows, F] -> [128, (C_rows/128) * F] SBUF layout (partition-major)."""
    from ml_dtypes import bfloat16

    rows, f = a.shape
    nck = rows // 128
    return np.ascontiguousarray(
        a.reshape(nck, 128, f).transpose(1, 0, 2).reshape(128, nck * f)
    ).astype(bfloat16)


def _prep_inputs(x, ve, cos, sin, Wq, Wk, Wv, Wproj, Wgate, W):
    from ml_dtypes import bfloat16

    cosT = cos[0, :, 0, :].T  # (64, T)
    sinT = sin[0, :, 0, :].T
    cc = np.concatenate([cosT, cosT], axis=0).astype(bfloat16)
    # rows 0:64 = -sin (used by tmp[64:128] = x1 * -sin, both operands base 0),
    # rows 64:128 = sin (used by tmp[0:64] = x2 * sin, both operands base 64).
    ss = np.concatenate([-sinT, sinT], axis=0).astype(bfloat16)
    # Additive -100 masks, pre-transposed for use as the matmul stationary
    # (out[p,f] += Mx^T[f,p] via an identity moving operand):
    #   causal block valid iff p <= f; edge block valid iff f <= p + W%128.
    p = np.arange(128)[:, None]
    f = np.arange(128)[None, :]
    mc = np.where(p <= f, 0.0, -100.0).astype(np.float32).T
    me = np.where(f <= p + (W % 128), 0.0, -100.0).astype(np.float32).T
    ident = np.eye(128, dtype=np.float32)
    tri = np.concatenate([mc, me, ident], axis=1).astype(bfloat16)

    xp_by_b = {}
    for b in range(B):
        xp_by_b[b] = _pack_rows(x[b].T)  # [128, NCK*T]

    in_maps = []
    for core in range(8):
        b, g = core // NKV, core % NKV
        hs = slice(g * HPC * HD, (g + 1) * HPC * HD)
        ks = slice(g * HD, (g + 1) * HD)
        gate = 3.0 / (1.0 + np.exp(-(x[b][:, :GATE_CH] @ Wgate[g])))  # (T,)
        vep = gate[:, None] * ve[b][:, ks]  # (T, HD)
        in_maps.append({
            "xp": xp_by_b[b],
            "wqp": _pack_rows(Wq[hs, :].T),
            "wkp": _pack_rows(Wk[ks, :].T),
            "wvp": _pack_rows(Wv[ks, :].T),
            "wpp": _pack_rows(Wproj[:, hs].T),
            "cc": cc,
            "ss": ss,
            "vep": _pack_rows(vep),
            "tri": tri,
        })
    return in_maps


def _run(inputs, trace=False):
    from concourse.bass_utils import run_bass_kernel_spmd

    x = np.asarray(inputs["x"], dtype=np.float32)
    ve = np.asarray(inputs["ve"], dtype=np.float32)
    cos = np.asarray(inputs["cos"], dtype=np.float32)
    sin = np.asarray(inputs["sin"], dtype=np.float32)
    Wq = np.asarray(inputs["Wq"], dtype=np.float32)
    Wk = np.asarray(inputs["Wk"], dtype=np.float32)
    Wv = np.asarray(inputs["Wv"], dtype=np.float32)
    Wproj = np.asarray(inputs["Wproj"], dtype=np.float32)
    Wgate = np.asarray(inputs["Wgate"], dtype=np.float32)
    W = int(inputs["window_size"])

    if W not in _compiled:
        _compiled[W] = _build(W)
    nc = _compiled[W]

    in_maps = _prep_inputs(x, ve, cos, sin, Wq, Wk, Wv, Wproj, Wgate, W)
    res = run_bass_kernel_spmd(nc, in_maps, core_ids=list(range(8)), trace=trace)

    out = np.zeros((B, T, C), dtype=np.float32)
    for core in range(8):
        b = core // NKV
        # outp: [128, NTT, T] with (p, co, t) = partial[co*128+p, t]
        part = np.asarray(res.results[core]["outp"]).astype(np.float32)
        out[b] += part.transpose(1, 0, 2).reshape(C, T).T
    return out, res


def kernel(**inputs):
    out, _ = _run(inputs, trace=False)
    return out
